# revision 1
# baseline (speedup 1.0000x reference)
"""Trainium2 Bass kernel for the CudaNorm FastWeight DPFP transformer layer.

Sharding: batch (8) across the 8 cores; each core runs its batch's full layer:
qkvb projection, DPFP feature maps, chunked delta-rule fast-weight scan
(C=128, depth-1 Neumann solve), output projection, residual + LayerNorm.

Self-contained: hardcodes all shapes; host-side prep rearranges weights and
builds masks/identity constants passed as extra DRAM inputs.
"""
import os
import numpy as np
import ml_dtypes

import concourse.bass as bass
import concourse.mybir as mybir
from concourse.bass_utils import run_bass_kernel_spmd
from concourse.tile import TileContext
from concourse.vector_clock import ScopedClock, VectorClock
from contextlib import ExitStack

F32 = mybir.dt.float32
BF16 = mybir.dt.bfloat16
AF = mybir.ActivationFunctionType
OP = mybir.AluOpType
AX = mybir.AxisListType

SLEN, BSZ, DM = 2048, 8, 1024
NH, DH, NROLL = 16, 64, 2
D = 2 * NROLL * DH            # 256 feature dim
C = 128                       # chunk length
NCH = SLEN // C               # 16 chunks
EPS, LN_EPS = 1e-5, 1e-5
SCALE = 1.0 / float(np.sqrt(DH))
OQKV = NH * 192               # 3072
OTOT = OQKV + NH              # 3088 (qkv + per-head b columns)

# ---------------------------------------------------------------- tile ctx
MAXW = 2


class PatchedTileContext(TileContext):
    """Work around walrus TPB sync-command limits: each instruction carries at
    most 2 sync commands (waits+updates); hoist excess waits onto preceding
    same-engine NoOps (1 wait each), and emit the kernel-tail drain's waits
    one-per-nop on SP."""

    def _lower_ordered_insts(self, ordered):
        for bb_name in list(ordered.keys()):
            new = []
            for inst in ordered[bb_name]:
                si = inst.sync_info
                nupd = len(si.on_update) if si is not None and si.on_update else 0
                maxw = max(0, MAXW - nupd)
                if si is not None and si.on_wait and len(si.on_wait) > maxw:
                    waits = list(si.on_wait)
                    excess = waits if maxw == 0 else waits[:-maxw]
                    keep = [] if maxw == 0 else waits[-maxw:]
                    for w in excess:
                        nop = mybir.InstNoOp(
                            name=self.nc.get_next_instruction_name(),
                            engine=inst.engine, ins=[], outs=[])
                        nop.sync_info = mybir.SyncInfo(on_wait=[w], on_update=[])
                        new.append(nop)
                    inst.sync_info = mybir.SyncInfo(
                        on_wait=keep, on_update=list(si.on_update or []))
                new.append(inst)
            ordered[bb_name] = new
        return super()._lower_ordered_insts(ordered)

    def _drain_and_barrier(self, tick_clock, wait_clock):
        gc = tick_clock.global_clock
        n = len(gc)
        for p in range(n):
            if gc[p] > 0:
                vc = VectorClock([gc[i] if i == p else 0 for i in range(n)])
                nop = self.nc.sync.nop(nofuse=True)
                wait_clock.add_sem_waits(nop.ins, ScopedClock({None: vc}))
        self.nc.sync.drain()
        self.nc.all_engine_barrier()
        assert self.sems is not None
        popped = self.nc._tile_sem_poison_stack.pop()
        assert popped is self._sem_poison
        self.nc.clear_and_free_semaphores(list(self.sems.allocated().values()))
        self.nc.all_engine_barrier()


# ---------------------------------------------------------------- program
def build_program(n_chunks=NCH, n_heads=NH):
    nc = bass.Bass()
    d_hT = nc.declare_dram_parameter("hT", [DM, SLEN], BF16, isOutput=False)
    d_hres = nc.declare_dram_parameter("hres", [SLEN, DM], F32, isOutput=False)
    d_w = nc.declare_dram_parameter("wqkv", [DM, OTOT], BF16, isOutput=False)
    d_wo = nc.declare_dram_parameter("woT", [DM, DM], BF16, isOutput=False)
    d_lng = nc.declare_dram_parameter("lng", [128, DM], F32, isOutput=False)
    d_lnb = nc.declare_dram_parameter("lnb", [128, DM], F32, isOutput=False)
    d_mSL = nc.declare_dram_parameter("maskSL", [128, 132], F32, isOutput=False)
    d_mLI = nc.declare_dram_parameter("maskLI", [128, 132], F32, isOutput=False)
    d_mUI = nc.declare_dram_parameter("maskUI", [128, 132], F32, isOutput=False)
    d_id = nc.declare_dram_parameter("identb", [128, 128], BF16, isOutput=False)
    d_out = nc.declare_dram_parameter("out", [SLEN, DM], F32, isOutput=True)

    with PatchedTileContext(nc) as tc, ExitStack() as ctx:
        # ---- persistent pools (bufs=1)
        P = lambda name, bufs, **kw: ctx.enter_context(
            tc.tile_pool(name=name, bufs=bufs, **kw))
        const = P("const", 1)
        state = P("state", 1)
        # ---- streaming pools
        import os as _os
        B = lambda k, d: int(_os.environ.get(k, d))
        hts_p = P("hts", 2)
        raw_p = P("raw", 2)
        sig_p = P("sig", 2)
        cols_p = P("cols", 2)
        feat_p = P("feat", 1)     # K1T/Q1T/Ktd per chunk (all heads)
        ftmp_p = P("ftmp", B("FTB", 6))     # xp, f fp32 temps
        sc_p = P("sc", B("SCB", 8))         # scan sbuf temps (N', B', ...)
        bd_p = P("bd", B("BDB", 8))         # bf16 [128,128] scratch for stt outs
        outT_p = P("outT", 2)
        xln_p = P("xln", 2)
        # PSUM budget: 8 banks total -> blk(3) + tp(2) + s(2) + pAT(1)
        psA_p = P("psA", B("BLKB", 3), space="PSUM")      # proj/A/S1/S2 rotating (tag blk)
        psT_p = P("psT", B("TPB", 2), space="PSUM")      # transpose outs (tag tp)
        psS_p = P("psS", B("SB", 2), space="PSUM")      # solve/state (tag s)
        psO_p = P("psO", 1, space="PSUM")      # out-proj (tag pAT)

        # ---- constants
        t_mSL = const.tile([128, 132], F32, tag="mSL", name="mSL"); nc.sync.dma_start(t_mSL[:], d_mSL[:])
        t_mLI = const.tile([128, 132], F32, tag="mLI", name="mLI"); nc.sync.dma_start(t_mLI[:], d_mLI[:])
        t_mUI = const.tile([128, 132], F32, tag="mUI", name="mUI"); nc.sync.dma_start(t_mUI[:], d_mUI[:])
        t_id = const.tile([128, 128], BF16, tag="id", name="id"); nc.sync.dma_start(t_id[:], d_id[:])
        t_lng = const.tile([128, DM], F32, tag="lng", name="lng"); nc.sync.dma_start(t_lng[:], d_lng[:])
        t_lnb = const.tile([128, DM], F32, tag="lnb", name="lnb"); nc.sync.dma_start(t_lnb[:], d_lnb[:])
        t_w = []
        for mc in range(8):
            t = const.tile([128, OTOT], BF16, tag=f"w{mc}", name=f"w{mc}")
            nc.sync.dma_start(t[:], d_w[mc * 128:(mc + 1) * 128, :])
            t_w.append(t)
        t_wo = []
        for ic in range(8):
            t = const.tile([128, DM], BF16, tag=f"wo{ic}", name=f"wo{ic}")
            nc.sync.dma_start(t[:], d_wo[ic * 128:(ic + 1) * 128, :])
            t_wo.append(t)

        # ---- state: per-head W ([128, 128]: dc0 cols 0:64, dc1 64:128), r
        t_Wm, t_Wb = [], []
        for hd in range(n_heads):
            wm = state.tile([128, 128], F32, tag=f"wm{hd}", name=f"wm{hd}")
            nc.vector.memset(wm[:], 0.0)
            wb = state.tile([128, 128], BF16, tag=f"wb{hd}", name=f"wb{hd}")
            nc.vector.memset(wb[:], 0.0)
            t_Wm.append(wm); t_Wb.append(wb)
        t_r = []
        for dc in range(2):
            r = state.tile([128, NH], F32, tag=f"r{dc}", name=f"r{dc}")
            nc.vector.memset(r[:], 0.0)
            t_r.append(r)

        for c in range(n_chunks):
            cs = slice(c * 128, (c + 1) * 128)
            # ================= projection: out[t, o] for this chunk =========
            hts = hts_p.tile([128, 1024], BF16, tag="hts", name="hts")
            for mc in range(8):
                nc.sync.dma_start(hts[:, mc * 128:(mc + 1) * 128],
                                  d_hT[mc * 128:(mc + 1) * 128, cs])
            raw = raw_p.tile([128, OTOT], BF16, tag="raw", name="raw")
            ogs = [(i * 512, 512) for i in range(6)] + [(OQKV, NH)]
            for (o0, ow) in ogs:
                pg = psA_p.tile([128, ow], F32, tag="blk", name="blk")
                for mc in range(8):
                    nc.tensor.matmul(pg[:], hts[:, mc * 128:(mc + 1) * 128],
                                     t_w[mc][:, o0:o0 + ow],
                                     start=(mc == 0), stop=(mc == 7))
                nc.vector.tensor_copy(raw[:, o0:o0 + ow], pg[:])
            sig = sig_p.tile([128, NH], F32, tag="sig", name="sig")
            nc.scalar.activation(sig[:], raw[:, OQKV:OQKV + NH], AF.Sigmoid)

            # ================= features (all heads) =========================
            K1T, Q1T, Ktd = [], [], []
            for hd in range(n_heads):
                k1t = [feat_p.tile([128, 132], BF16, tag=f"k1t{hd}_{dc}", name=f"k1t{hd}_{dc}")
                       for dc in range(2)]
                q1t = [feat_p.tile([128, 128], BF16, tag=f"q1t{hd}_{dc}", name=f"q1t{hd}_{dc}")
                       for dc in range(2)]
                ktd = feat_p.tile([128, 256], BF16, tag=f"ktd{hd}", name=f"ktd{hd}")
                K1T.append(k1t); Q1T.append(q1t); Ktd.append(ktd)
                qoff = hd * 192
                for (src_off, is_k) in ((qoff, 0), (qoff + 64, 1)):
                    xp = ftmp_p.tile([128, 128], F32, tag="xp", name="xp")
                    nc.scalar.activation(xp[:, 0:64], raw[:, src_off:src_off + 64],
                                         AF.Relu)
                    nc.scalar.activation(xp[:, 64:128], raw[:, src_off:src_off + 64],
                                         AF.Relu, scale=-1.0)
                    f = ftmp_p.tile([128, 256], F32, tag="f", name="f")
                    nc.vector.tensor_mul(f[:, 1:128], xp[:, 1:128], xp[:, 0:127])
                    nc.vector.tensor_mul(f[:, 0:1], xp[:, 0:1], xp[:, 127:128])
                    nc.vector.tensor_mul(f[:, 130:256], xp[:, 2:128], xp[:, 0:126])
                    nc.vector.tensor_mul(f[:, 128:130], xp[:, 0:2], xp[:, 126:128])
                    fsum = ftmp_p.tile([128, 1], F32, tag="fsum", name="fsum")
                    nc.vector.tensor_reduce(fsum[:], f[:], AX.X, OP.add)
                    frec = ftmp_p.tile([128, 1], F32, tag="frec", name="frec")
                    nc.vector.reciprocal(frec[:], fsum[:])
                    if is_k:
                        td = ktd
                    else:
                        td = ftmp_p.tile([128, 256], BF16, tag="qtd", name="qtd")
                    nc.scalar.mul(td[:], f[:], frec[:])
                    dst = k1t if is_k else q1t
                    for dc in range(2):
                        pt = psT_p.tile([128, 128], BF16, tag="tp", name="tp")
                        nc.tensor.transpose(pt[:], td[:, dc * 128:(dc + 1) * 128],
                                            t_id[:])
                        nc.vector.tensor_copy(dst[dc][:, 0:128], pt[:])
                for dc in range(2):
                    nc.gpsimd.tensor_copy(k1t[dc][:, 128:129],
                                          t_r[dc][:, hd:hd + 1])

            # ================= scan pass 1: block-diag matmuls ==============
            kd_all = cols_p.tile([128, NH], F32, tag="kd", name="kd")
            dn_all = cols_p.tile([128, NH], F32, tag="dn", name="dn")
            Shat, Araw = [], []
            for hd in range(n_heads):
                pA = psA_p.tile([128, 132], F32, tag="blk", name="blk")
                pS1 = psA_p.tile([128, 128], F32, tag="blk", name="blk")
                pS2 = psA_p.tile([128, 132], F32, tag="blk", name="blk")
                for dc in range(2):
                    st = (dc == 0); sp = (dc == 1)
                    nc.tensor.matmul(pA[:, 0:129], K1T[hd][dc][:, 0:128],
                                     K1T[hd][dc][:, 0:129], start=st, stop=sp)
                    nc.tensor.matmul(pS1[:], K1T[hd][dc][:, 0:128],
                                     Q1T[hd][dc][:], start=st, stop=sp)
                    nc.tensor.matmul(pS2[:, 0:129], Q1T[hd][dc][:],
                                     K1T[hd][dc][:, 0:129], start=st, stop=sp)
                scr = bd_p.tile([128, 132], BF16, tag="scr", name="scr")
                nc.vector.scalar_tensor_tensor(scr[:, 0:129], pA[:, 0:129], 1.0,
                                               t_mSL[:, 0:129], OP.mult, OP.mult,
                                               accum_out=kd_all[:, hd:hd + 1])
                if c == 0:
                    nc.vector.memset(kd_all[0:1, hd:hd + 1], 1.0)
                scr2 = bd_p.tile([128, 132], BF16, tag="scr", name="scr")
                nc.vector.scalar_tensor_tensor(scr2[:, 0:129], pS2[:, 0:129], 1.0,
                                               t_mLI[:, 0:129], OP.mult, OP.mult,
                                               accum_out=dn_all[:, hd:hd + 1])
                sh = sc_p.tile([128, 128], BF16, tag=f"sh{hd % 4}", name=f"sh{hd % 4}")
                nc.vector.tensor_mul(sh[:], pS1[:], t_mUI[:, 0:128])
                ar = sc_p.tile([128, 128], BF16, tag=f"ar{hd % 4}", name=f"ar{hd % 4}")
                nc.vector.tensor_copy(ar[:], pA[:, 0:128])
                Shat.append(sh); Araw.append(ar)

            # ================= chunk-level columns ==========================
            ceps = cols_p.tile([128, NH], F32, tag="ceps", name="ceps")
            nc.vector.tensor_scalar_add(ceps[:], kd_all[:], EPS)
            c_all = cols_p.tile([128, NH], F32, tag="c", name="c")
            nc.vector.reciprocal(c_all[:], ceps[:])
            cneg = cols_p.tile([128, NH], F32, tag="cneg", name="cneg")
            nc.vector.tensor_scalar_mul(cneg[:], c_all[:], -1.0)
            t0 = cols_p.tile([128, NH], F32, tag="t0", name="t0")
            nc.vector.tensor_mul(t0[:], kd_all[:], c_all[:])
            cb_all = cols_p.tile([128, NH], F32, tag="cb", name="cb")
            nc.vector.tensor_mul(cb_all[:], t0[:], sig[:])
            cbc = cols_p.tile([128, NH], F32, tag="cbc", name="cbc")
            nc.vector.tensor_mul(cbc[:], cb_all[:], c_all[:])
            dne = cols_p.tile([128, NH], F32, tag="dne", name="dne")
            nc.vector.tensor_scalar_add(dne[:], dn_all[:], EPS)
            dnr = cols_p.tile([128, NH], F32, tag="dnr", name="dnr")
            nc.vector.reciprocal(dnr[:], dne[:])
            dnrS = cols_p.tile([128, NH], F32, tag="dnrS", name="dnrS")
            nc.vector.tensor_scalar_mul(dnrS[:], dnr[:], SCALE)

            # ================= scan pass 2: solve + outputs + W =============
            outT = [outT_p.tile([128, 128], BF16, tag=f"oT{ic}", name=f"oT{ic}")
                    for ic in range(n_heads // 2)]
            for hd in range(n_heads):
                voff = hd * 192 + 128
                # N'' and B'' = N''^T
                Nt = sc_p.tile([128, 128], BF16, tag="Nt", name="Nt")
                nc.vector.scalar_tensor_tensor(Nt[:], Araw[hd][:], cbc[:, hd:hd + 1],
                                               t_mSL[:, 0:128], OP.mult, OP.mult)
                pB = psT_p.tile([128, 128], BF16, tag="tp", name="tp")
                nc.tensor.transpose(pB[:], Nt[:], t_id[:])
                Bt = sc_p.tile([128, 128], BF16, tag="Bt", name="Bt")
                nc.vector.tensor_copy(Bt[:], pB[:])
                pB2 = psS_p.tile([128, 128], F32, tag="s", name="s")
                nc.tensor.matmul(pB2[:], Nt[:], Bt[:], start=True, stop=True)
                B2t = sc_p.tile([128, 128], BF16, tag="B2t", name="B2t")
                nc.vector.tensor_copy(B2t[:], pB2[:])
                # X0 = cb*(V - c*KW)
                X0 = sc_p.tile([128, 64], BF16, tag="X0", name="X0")
                if c == 0:
                    nc.scalar.mul(X0[:], raw[:, voff:voff + 64], cb_all[:, hd:hd + 1])
                else:
                    pKW = psS_p.tile([128, 64], F32, tag="s", name="s")
                    for dc in range(2):
                        nc.tensor.matmul(pKW[:], K1T[hd][dc][:, 0:128],
                                         t_Wb[hd][:, dc * 64:(dc + 1) * 64],
                                         start=(dc == 0), stop=(dc == 1))
                    x0f = ftmp_p.tile([128, 64], F32, tag="x0f", name="x0f")
                    nc.vector.scalar_tensor_tensor(x0f[:], pKW[:], cneg[:, hd:hd + 1],
                                                   raw[:, voff:voff + 64],
                                                   OP.mult, OP.add)
                    nc.scalar.mul(X0[:], x0f[:], cb_all[:, hd:hd + 1])
                # X1 = X0 + N''^2 X0 ; Y = X1 - N'' X1
                pX = psS_p.tile([128, 64], F32, tag="s", name="s")
                nc.tensor.matmul(pX[:], B2t[:], X0[:], start=True, stop=True)
                X1 = sc_p.tile([128, 64], BF16, tag="X1", name="X1")
                nc.vector.tensor_add(X1[:], pX[:], X0[:])
                pY = psS_p.tile([128, 64], F32, tag="s", name="s")
                nc.tensor.matmul(pY[:], Bt[:], X1[:], start=True, stop=True)
                Yt = sc_p.tile([128, 64], BF16, tag="Yt", name="Yt")
                nc.vector.tensor_sub(Yt[:], X1[:], pY[:])
                # Out = QW + tril(S1) Y, scaled by SCALE/(denom+eps)
                pOut = psS_p.tile([128, 64], F32, tag="s", name="s")
                if c == 0:
                    nc.tensor.matmul(pOut[:], Shat[hd][:], Yt[:],
                                     start=True, stop=True)
                else:
                    for dc in range(2):
                        nc.tensor.matmul(pOut[:], Q1T[hd][dc][:],
                                         t_Wb[hd][:, dc * 64:(dc + 1) * 64],
                                         start=(dc == 0), stop=False)
                    nc.tensor.matmul(pOut[:], Shat[hd][:], Yt[:],
                                     start=False, stop=True)
                outc = sc_p.tile([128, 64], BF16, tag="outc", name="outc")
                nc.scalar.mul(outc[:], pOut[:], dnrS[:, hd:hd + 1])
                pT = psT_p.tile([128, 128], BF16, tag="tp", name="tp")
                base = (hd % 2) * 64
                nc.tensor.transpose(pT[base:base + 64, :], outc[:], t_id[:],
                                    tile_position=(0, base))
                nc.vector.tensor_copy(outT[hd // 2][base:base + 64, :],
                                      pT[base:base + 64, :])
                # W update: W += K1^T Y  (via Ktd), r update
                pW = psS_p.tile([128, 128], F32, tag="s", name="s")
                for dc in range(2):
                    nc.tensor.matmul(pW[:, dc * 64:(dc + 1) * 64],
                                     Ktd[hd][:, dc * 128:(dc + 1) * 128], Yt[:],
                                     start=True, stop=True)
                nc.vector.tensor_add(t_Wm[hd][:], pW[:], t_Wm[hd][:])
                nc.gpsimd.tensor_copy(t_Wb[hd][:], t_Wm[hd][:])
                for dc in range(2):
                    rs = ftmp_p.tile([128, 1], F32, tag="rs", name="rs")
                    nc.vector.tensor_reduce(rs[:], K1T[hd][dc][:, 0:128], AX.X,
                                            OP.add)
                    nc.vector.tensor_add(t_r[dc][:, hd:hd + 1],
                                         t_r[dc][:, hd:hd + 1], rs[:])

            # ================= output projection + residual + LN ============
            hr = xln_p.tile([128, DM], F32, tag="hr", name="hr")
            nc.sync.dma_start(hr[:], d_hres[cs, :])
            x = xln_p.tile([128, DM], F32, tag="x", name="x")
            for og in range(2):
                pAT = psO_p.tile([128, 512], F32, tag="pAT", name="pAT")
                for ic in range(n_heads // 2):
                    nc.tensor.matmul(pAT[:], outT[ic][:],
                                     t_wo[ic][:, og * 512:(og + 1) * 512],
                                     start=(ic == 0), stop=(ic == n_heads // 2 - 1))
                nc.vector.tensor_add(x[:, og * 512:(og + 1) * 512], pAT[:],
                                     hr[:, og * 512:(og + 1) * 512])
            xsum = ftmp_p.tile([128, 1], F32, tag="xsum", name="xsum")
            nc.vector.tensor_reduce(xsum[:], x[:], AX.X, OP.add)
            nmu = ftmp_p.tile([128, 1], F32, tag="nmu", name="nmu")
            nc.vector.tensor_scalar_mul(nmu[:], xsum[:], -1.0 / DM)
            nc.vector.tensor_scalar_add(x[:], x[:], nmu[:])
            vscr = xln_p.tile([128, DM], BF16, tag="vscr", name="vscr")
            var = ftmp_p.tile([128, 1], F32, tag="var", name="var")
            nc.vector.scalar_tensor_tensor(vscr[:], x[:], 1.0, x[:],
                                           OP.mult, OP.mult, accum_out=var[:])
            vare = ftmp_p.tile([128, 1], F32, tag="vare", name="vare")
            nc.vector.tensor_scalar(vare[:], var[:], 1.0 / DM, float(LN_EPS),
                                    OP.mult, OP.add)
            sd = ftmp_p.tile([128, 1], F32, tag="sd", name="sd")
            nc.scalar.sqrt(sd[:], vare[:])
            rstd = ftmp_p.tile([128, 1], F32, tag="rstd", name="rstd")
            nc.vector.reciprocal(rstd[:], sd[:])
            nc.vector.scalar_tensor_tensor(x[:], x[:], rstd[:], t_lng[:],
                                           OP.mult, OP.mult)
            nc.vector.tensor_add(x[:], x[:], t_lnb[:])
            nc.sync.dma_start(d_out[cs, :], x[:])

    return nc


# ---------------------------------------------------------------- host side
def _prep_core_inputs(h_b, W_qkvb, W_o, ln_g, ln_b):
    bf16 = ml_dtypes.bfloat16
    hT = np.ascontiguousarray(h_b.T).astype(bf16)                  # [1024, 2048]
    wq = np.zeros((DM, OTOT), dtype=bf16)
    Wr = W_qkvb.reshape(NH, 193, DM)
    for hd in range(NH):
        wq[:, hd * 192:hd * 192 + 192] = Wr[hd, 0:192, :].T
        wq[:, OQKV + hd] = Wr[hd, 192, :]
    woT = np.ascontiguousarray(W_o.T).astype(bf16)                 # [i, o]
    lng = np.broadcast_to(ln_g[None, :], (128, DM)).astype(np.float32).copy()
    lnb = np.broadcast_to(ln_b[None, :], (128, DM)).astype(np.float32).copy()
    ii, jj = np.indices((128, 132))
    mSL = (jj < ii).astype(np.float32);  mSL[:, 128] = 1.0
    mLI = (jj <= ii).astype(np.float32); mLI[:, 128] = 1.0
    mUI = ((jj >= ii) & (jj < 128)).astype(np.float32)
    identb = np.eye(128, dtype=bf16)
    return {"hT": hT, "hres": np.ascontiguousarray(h_b, np.float32),
            "wqkv": wq, "woT": woT, "lng": lng, "lnb": lnb,
            "maskSL": mSL, "maskLI": mLI, "maskUI": mUI, "identb": identb}


_cached = {}


def kernel(h, W_qkvb, W_o, ln_g, ln_b):
    h = np.asarray(h, np.float32)
    W_qkvb = np.asarray(W_qkvb, np.float32)
    W_o = np.asarray(W_o, np.float32)
    ln_g = np.asarray(ln_g, np.float32)
    ln_b = np.asarray(ln_b, np.float32)
    if "nc" not in _cached:
        _cached["nc"] = build_program()
    nc = _cached["nc"]
    in_maps = [_prep_core_inputs(h[:, b, :], W_qkvb, W_o, ln_g, ln_b)
               for b in range(BSZ)]
    res = run_bass_kernel_spmd(nc, in_maps, list(range(BSZ)),
                               trace=os.environ.get("BASS_TRACE", "") == "1")
    out = np.stack([res.results[b]["out"] for b in range(BSZ)], axis=1)
    kernel.last_exec_time_ns = res.exec_time_ns
    return out.astype(np.float32)



# revision 2
# speedup vs baseline: 1.1468x; 1.1468x over previous
"""Trainium2 Bass kernel v2: head-batched CudaNorm FastWeight DPFP layer.

Batch sharded across 8 cores (1 batch element per core). Per chunk (C=128):
qkvb projection (PSUM-wide groups), DPFP features batched over all 16 heads
via strided 3D APs, merged Gram matmuls ([A|kr|S1] + [S2|qr] per head in one
PSUM bank), batched mask/reduce extraction, 3-matmul Neumann solve per head
(Y = (I-N)(I+N^2)X0 via t1=NX0, t2=N*t1, pY=N*X1), transpose-free Shat,
single all-heads W state with one Pool copy per chunk.
"""
import os
import numpy as np
import ml_dtypes

import concourse.bass as bass
import concourse.mybir as mybir
from concourse.bass_utils import run_bass_kernel_spmd
from concourse.tile import TileContext
from concourse.vector_clock import ScopedClock, VectorClock
from contextlib import ExitStack

F32 = mybir.dt.float32
BF16 = mybir.dt.bfloat16
FP8 = mybir.dt.float8e4
WSC = 32.0
AF = mybir.ActivationFunctionType
OP = mybir.AluOpType
AX = mybir.AxisListType

SLEN, BSZ, DM = 2048, 8, 1024
NH, DH, NROLL = 16, 64, 2
D = 2 * NROLL * DH            # 256 feature dim (2 dc of 128)
C = 128                       # chunk length
NCH = SLEN // C               # 16 chunks
EPS, LN_EPS = 1e-5, 1e-5
SCALE = 1.0 / float(np.sqrt(DH))
OQKV = NH * 192               # 3072
OTOT = OQKV + NH              # 3088
KQW = 260                     # per-head stride in KQ tiles: K(128)|r(1)|Q(128)|pad

MAXW = 2


class PatchedTileContext(TileContext):
    """Work around walrus TPB sync-command limits: each instruction carries at
    most 2 sync commands (waits+updates); hoist excess waits onto preceding
    same-engine NoOps (1 wait each), and emit the kernel-tail drain's waits
    one-per-nop on SP."""

    def _lower_ordered_insts(self, ordered):
        for bb_name in list(ordered.keys()):
            new = []
            for inst in ordered[bb_name]:
                si = inst.sync_info
                nupd = len(si.on_update) if si is not None and si.on_update else 0
                maxw = max(0, MAXW - nupd)
                if si is not None and si.on_wait and len(si.on_wait) > maxw:
                    waits = list(si.on_wait)
                    excess = waits if maxw == 0 else waits[:-maxw]
                    keep = [] if maxw == 0 else waits[-maxw:]
                    for w in excess:
                        nop = mybir.InstNoOp(
                            name=self.nc.get_next_instruction_name(),
                            engine=inst.engine, ins=[], outs=[])
                        nop.sync_info = mybir.SyncInfo(on_wait=[w], on_update=[])
                        new.append(nop)
                    inst.sync_info = mybir.SyncInfo(
                        on_wait=keep, on_update=list(si.on_update or []))
                new.append(inst)
            ordered[bb_name] = new
        return super()._lower_ordered_insts(ordered)

    def _drain_and_barrier(self, tick_clock, wait_clock):
        gc = tick_clock.global_clock
        n = len(gc)
        for p in range(n):
            if gc[p] > 0:
                vc = VectorClock([gc[i] if i == p else 0 for i in range(n)])
                nop = self.nc.sync.nop(nofuse=True)
                wait_clock.add_sem_waits(nop.ins, ScopedClock({None: vc}))
        self.nc.sync.drain()
        self.nc.all_engine_barrier()
        assert self.sems is not None
        popped = self.nc._tile_sem_poison_stack.pop()
        assert popped is self._sem_poison
        self.nc.clear_and_free_semaphores(list(self.sems.allocated().values()))
        self.nc.all_engine_barrier()


def _r3(ap, h):
    return ap.rearrange("p (h o) -> p h o", h=h)


def _b3(ap, n, w):
    # [128, n] -> [128, n, w] with stride-0 inner axis
    return ap.unsqueeze(-1).broadcast_to([128, n, w])


# ---------------------------------------------------------------- program
def build_program(n_chunks=NCH):
    nc = bass.Bass()
    d_hT = nc.declare_dram_parameter("hT", [DM, SLEN], FP8, isOutput=False)
    d_hres = nc.declare_dram_parameter("hres", [SLEN, DM], F32, isOutput=False)
    d_w = nc.declare_dram_parameter("wqkv", [DM, OTOT], FP8, isOutput=False)
    d_wo = nc.declare_dram_parameter("woT", [DM, DM], BF16, isOutput=False)
    d_lng = nc.declare_dram_parameter("lng", [128, DM], BF16, isOutput=False)
    d_lnb = nc.declare_dram_parameter("lnb", [128, DM], BF16, isOutput=False)
    d_mSL = nc.declare_dram_parameter("maskSL", [128, 132], F32, isOutput=False)
    d_mUI = nc.declare_dram_parameter("maskUI", [128, 132], F32, isOutput=False)
    d_out = nc.declare_dram_parameter("out", [SLEN, DM], F32, isOutput=True)

    with PatchedTileContext(nc) as tc, ExitStack() as ctx:
        P = lambda name, bufs, **kw: ctx.enter_context(
            tc.tile_pool(name=name, bufs=bufs, **kw))
        const = P("const", 1)
        state = P("state", 1)
        hts_p = P("hts", 1)
        raw_p = P("raw", 1)
        sig_p = P("sig", 1)
        fx_p = P("fx", 1)      # one [128,2048] bf16 tag, rotated q/k
        ff_p = P("ff", 1)      # one [128,2048] f32 tag, rotated 4x (q/k halves)
        fn_p = P("fn", 2)
        kq_p = P("kq", 2)
        scr_p = P("scr", 1)
        cols_p = P("cols", 1)
        sc_p = P("sc", 1)
        oc_p = P("oc", 1)
        ftmp_p = P("ftmp", 4)
        xln_p = P("xln", 1)
        psG_p = P("psG", 1, space="PSUM")   # 4 banks: gram groups / projection
        psS_p = P("psS", 4, space="PSUM")   # 4 banks: solve/outproj/pW

        # ---- constants
        t_mSL = const.tile([128, 132], F32, tag="mSL", name="mSL"); nc.sync.dma_start(t_mSL[:], d_mSL[:])
        t_mUI = const.tile([128, 132], F32, tag="mUI", name="mUI"); nc.sync.dma_start(t_mUI[:], d_mUI[:])
        t_lng = const.tile([128, DM], BF16, tag="lng", name="lng"); nc.sync.dma_start(t_lng[:], d_lng[:])
        t_lnb = const.tile([128, DM], BF16, tag="lnb", name="lnb"); nc.sync.dma_start(t_lnb[:], d_lnb[:])
        tw_all = const.tile([128, 8 * OTOT], FP8, tag="tw", name="tw")
        nc.sync.dma_start(
            _r3(tw_all[:], 8),
            d_w[:].rearrange("(mc p) o -> mc p o", mc=8).transpose([1, 0, 2]))
        tw4 = _r3(tw_all[:], 8)
        t_wo = []
        for ic in range(8):
            t = const.tile([128, DM], BF16, tag=f"wo{ic}", name=f"wo{ic}")
            nc.sync.dma_start(t[:], d_wo[ic * 128:(ic + 1) * 128, :])
            t_wo.append(t)

        # ---- state (Wb per head: [Wdc0(64) | r0(1) | Wdc1(64) | r1(1)])
        t_Wm = state.tile([128, NH * 128], F32, tag="Wm", name="Wm")
        nc.vector.memset(t_Wm[:], 0.0)
        t_Wb = state.tile([128, NH * 130], BF16, tag="Wb", name="Wb")
        nc.vector.memset(t_Wb[:], 0.0)
        t_r = []
        for dc in range(2):
            r = state.tile([128, NH], F32, tag=f"r{dc}", name=f"r{dc}")
            nc.vector.memset(r[:], 0.0)
            t_r.append(r)

        for c in range(n_chunks):
            cs = slice((c % NCH) * 128, (c % NCH) * 128 + 128)
            # ================= projection =================================
            hts = hts_p.tile([128, 1024], FP8, tag="hts", name="hts")
            src = d_hT[:, cs].rearrange("(mc p) t -> mc p t", mc=8).transpose([1, 0, 2])
            nc.sync.dma_start(_r3(hts[:], 8), src)
            raw = raw_p.tile([128, OTOT], BF16, tag="raw", name="raw")
            pg = psG_p.tile([128, 2048], F32, tag="G", name="G")
            for g in range(4):
                for m2 in range(4):
                    nc.tensor.matmul(
                        pg[:, g * 512:(g + 1) * 512],
                        hts[:, m2 * 256:(m2 + 1) * 256].rearrange(
                            "p (two f) -> p two f", two=2),
                        tw4[:, 2 * m2:2 * m2 + 2, g * 512:(g + 1) * 512],
                        start=(m2 == 0), stop=(m2 == 3),
                        perf_mode=mybir.MatmulPerfMode.DoubleRow)
            nc.vector.tensor_scalar_mul(raw[:, 0:2048], pg[:], 1.0 / WSC)
            pg2 = psG_p.tile([128, 2048], F32, tag="G", name="G")
            for gi, (o0, ow) in enumerate([(2048, 512), (2560, 512), (OQKV, NH)]):
                for m2 in range(4):
                    nc.tensor.matmul(
                        pg2[:, gi * 512:gi * 512 + ow],
                        hts[:, m2 * 256:(m2 + 1) * 256].rearrange(
                            "p (two f) -> p two f", two=2),
                        tw4[:, 2 * m2:2 * m2 + 2, o0:o0 + ow],
                        start=(m2 == 0), stop=(m2 == 3),
                        perf_mode=mybir.MatmulPerfMode.DoubleRow)
            nc.vector.tensor_scalar_mul(
                _r3(raw[:, 2048:OTOT].rearrange("p (a b) -> p a b", a=1)
                    .squeeze(1), 1),
                pg2[:, 0:1040].rearrange("p (a b) -> p a b", a=1).squeeze(1),
                1.0 / WSC)
            sig = sig_p.tile([128, NH], F32, tag="sig", name="sig")
            nc.scalar.activation(sig[:], raw[:, OQKV:OQKV + NH], AF.Sigmoid)

            # ================= features (all heads, strided) ==============
            # fqn/fkn are dc-major: [128, (2 dc, 16 h, 128)] so each dc block
            # is contiguous and transposes in ONE batched DMA transpose.
            rawq = _r3(raw[:, 0:OQKV], NH)  # [128, 16, 192]
            fqn = fn_p.tile([128, NH * 256], BF16, tag="fqn", name="fqn")
            fkn = fn_p.tile([128, NH * 256], BF16, tag="fkn", name="fkn")
            for which, (coff, fdst) in enumerate(((0, fqn), (64, fkn))):
                eng = nc.vector
                xp = fx_p.tile([128, NH * 128], BF16, tag="xp", name="xp")
                x3 = _r3(xp[:], NH)
                nc.scalar.activation(x3[:, :, 0:64], rawq[:, :, coff:coff + 64],
                                     AF.Relu)
                nc.scalar.activation(x3[:, :, 64:128],
                                     rawq[:, :, coff:coff + 64], AF.Relu,
                                     scale=-1.0)
                sums = ftmp_p.tile([128, NH], F32, tag=f"sums{which}",
                                   name=f"sums{which}")
                for half in range(2):
                    hh = slice(half * 8, (half + 1) * 8)
                    f = ff_p.tile([128, 8 * 256], BF16, tag="f", name="f")
                    # f dc-major: [128, (2 dc, 8 h, 128)]
                    f4 = f[:].rearrange("p (d h o) -> p d h o", d=2, h=8)
                    xh = x3[:, hh, :]
                    eng.tensor_mul(f4[:, 0, :, 1:128], xh[:, :, 1:128],
                                   xh[:, :, 0:127])
                    eng.tensor_mul(f4[:, 0, :, 0:1], xh[:, :, 0:1],
                                   xh[:, :, 127:128])
                    eng.tensor_mul(f4[:, 1, :, 2:128], xh[:, :, 2:128],
                                   xh[:, :, 0:126])
                    eng.tensor_mul(f4[:, 1, :, 0:2], xh[:, :, 0:2],
                                   xh[:, :, 126:128])
                    sr = f[:].rearrange("p (d h o) -> p h d o", d=2, h=8)
                    nc.vector.tensor_reduce(sums[:, hh], sr, AX.XY, OP.add)
                    rec = ftmp_p.tile([128, 8], F32, tag=f"rec{which}{half}",
                                      name=f"rec{which}{half}")
                    nc.vector.reciprocal(rec[:], sums[:, hh])
                    fd4 = fdst[:].rearrange("p (d h o) -> p d h o", d=2, h=NH)
                    recb = rec[:].unsqueeze(1).unsqueeze(-1).broadcast_to(
                        [128, 2, 8, 128])
                    eng.tensor_mul(fd4[:, :, hh, :], f4, recb)

            # ---- transposes into KQ tiles: per head [K(128)|r(1)|Q(128)|pad]
            KQ = [kq_p.tile([128, NH * KQW], BF16, tag=f"KQ{dc}", name=f"KQ{dc}")
                  for dc in range(2)]
            for dc in range(2):
                for which, fsrc, off in ((0, fkn, 0), (1, fqn, 129)):
                    stage = ff_p.tile([128, 2048], BF16, tag="stg", name="stg")
                    eng = nc.sync if (dc + which) % 2 == 0 else nc.scalar
                    eng.dma_start_transpose(
                        _r3(stage[:], NH),
                        fsrc[:, dc * NH * 128:(dc + 1) * NH * 128])
                    nc.scalar.copy(_r3(KQ[dc][:], NH)[:, :, off:off + 128],
                                   _r3(stage[:], NH))
                nc.vector.tensor_copy(
                    _r3(KQ[dc][:], NH)[:, :, 128:129],
                    t_r[dc][:].unsqueeze(-1))

            # ================= gram + extraction ==========================
            kd_all = cols_p.tile([128, NH], F32, tag="kd", name="kd")
            tmpA = scr_p.tile([128, NH * 129], BF16, tag="tmpA", name="tmpA")
            Shat = scr_p.tile([128, NH * 128], BF16, tag="Shat", name="Shat")
            for g in range(4):
                pgr = psG_p.tile([128, 2048], F32, tag="G", name="G")
                for j in range(4):
                    hd = g * 4 + j
                    base = hd * KQW
                    for dc in range(2):
                        nc.tensor.matmul(pgr[:, j * 512:j * 512 + 257],
                                         KQ[dc][:, base:base + 128],
                                         KQ[dc][:, base:base + 257],
                                         start=(dc == 0), stop=(dc == 1))
                pgr3 = _r3(pgr[:], 4)
                tmpA3 = _r3(tmpA[:], NH)[:, g * 4:(g + 1) * 4, :]
                nc.vector.tensor_mul(
                    tmpA3, pgr3[:, :, 0:129],
                    t_mSL[:, 0:129].unsqueeze(1).broadcast_to([128, 4, 129]))
                nc.vector.tensor_reduce(kd_all[:, g * 4:(g + 1) * 4], tmpA3,
                                        AX.X, OP.add)
                nc.vector.tensor_mul(
                    _r3(Shat[:], NH)[:, g * 4:(g + 1) * 4, :],
                    pgr3[:, :, 129:257],
                    t_mUI[:, 0:128].unsqueeze(1).broadcast_to([128, 4, 128]))
            if c == 0:
                nc.vector.memset(kd_all[0:1, :], 1.0)

            # ================= chunk columns ==============================
            ceps = cols_p.tile([128, NH], F32, tag="ceps", name="ceps")
            nc.vector.tensor_scalar_add(ceps[:], kd_all[:], EPS)
            c_all = cols_p.tile([128, NH], F32, tag="c", name="c")
            nc.vector.reciprocal(c_all[:], ceps[:])
            t0 = cols_p.tile([128, NH], F32, tag="t0", name="t0")
            nc.vector.tensor_mul(t0[:], kd_all[:], c_all[:])
            cb_all = cols_p.tile([128, NH], F32, tag="cb", name="cb")
            nc.vector.tensor_mul(cb_all[:], t0[:], sig[:])
            cbc = cols_p.tile([128, NH], F32, tag="cbc", name="cbc")
            nc.vector.tensor_mul(cbc[:], cb_all[:], c_all[:])
            cbcn = cols_p.tile([128, NH], F32, tag="cbcn", name="cbcn")
            nc.vector.tensor_scalar_mul(cbcn[:], cbc[:], -1.0)

            # ---- Nt (all heads) + Bt transposes
            Nt = scr_p.tile([128, NH * 128], BF16, tag="Nt", name="Nt")
            nc.vector.tensor_mul(_r3(Nt[:], NH),
                                 _r3(tmpA[:], NH)[:, :, 0:128],
                                 _b3(cbc[:], NH, 128))
            Bt = scr_p.tile([128, NH * 128], BF16, tag="Bt", name="Bt")
            nc.sync.dma_start_transpose(_r3(Bt[:], NH), Nt[:])

            # ================= solve + outputs + W ========================
            outc = oc_p.tile([128, NH * 64], BF16, tag="outc", name="outc")
            vall = oc_p.tile([128, NH * 64], BF16, tag="vall", name="vall")
            nc.scalar.copy(_r3(vall[:], NH), rawq[:, :, 128:192])
            Yts = []
            for g8 in range(2):
                hs = slice(g8 * 8, (g8 + 1) * 8)
                X0g = sc_p.tile([128, 512], BF16, tag=f"X0{g8}", name=f"X0{g8}")
                v3 = _r3(vall[:], NH)[:, hs, :]
                if c == 0:
                    nc.vector.tensor_mul(_r3(X0g[:], 8), v3,
                                         _b3(cb_all[:, hs], 8, 64))
                else:
                    pkw = psS_p.tile([128, 512], F32, tag="S", name="S")
                    for j in range(8):
                        hd = g8 * 8 + j
                        for dc in range(2):
                            nc.tensor.matmul(
                                pkw[:, j * 64:(j + 1) * 64],
                                KQ[dc][:, hd * KQW:hd * KQW + 128],
                                t_Wb[:, hd * 130 + dc * 65:hd * 130 + dc * 65 + 64],
                                start=(dc == 0), stop=(dc == 1))
                    xf = sc_p.tile([128, 512], F32, tag="xf", name="xf")
                    nc.vector.tensor_mul(_r3(xf[:], 8), _r3(pkw[:], 8),
                                         _b3(cbcn[:, hs], 8, 64))
                    xf2 = sc_p.tile([128, 512], F32, tag="xf2", name="xf2")
                    nc.vector.tensor_mul(_r3(xf2[:], 8), v3,
                                         _b3(cb_all[:, hs], 8, 64))
                    nc.vector.tensor_add(X0g[:], xf[:], xf2[:])
                pt1 = psS_p.tile([128, 512], F32, tag="S", name="S")
                for j in range(8):
                    hd = g8 * 8 + j
                    nc.tensor.matmul(pt1[:, j * 64:(j + 1) * 64],
                                     Bt[:, hd * 128:(hd + 1) * 128],
                                     X0g[:, j * 64:(j + 1) * 64],
                                     start=True, stop=True)
                t1s = sc_p.tile([128, 512], BF16, tag=f"t1s{g8}", name=f"t1s{g8}")
                nc.vector.tensor_copy(t1s[:], pt1[:])
                pt2 = psS_p.tile([128, 512], F32, tag="S", name="S")
                for j in range(8):
                    hd = g8 * 8 + j
                    nc.tensor.matmul(pt2[:, j * 64:(j + 1) * 64],
                                     Bt[:, hd * 128:(hd + 1) * 128],
                                     t1s[:, j * 64:(j + 1) * 64],
                                     start=True, stop=True)
                X1g = sc_p.tile([128, 512], BF16, tag=f"X1{g8}", name=f"X1{g8}")
                nc.vector.tensor_add(X1g[:], pt2[:], X0g[:])
                py = psS_p.tile([128, 512], F32, tag="S", name="S")
                for j in range(8):
                    hd = g8 * 8 + j
                    nc.tensor.matmul(py[:, j * 64:(j + 1) * 64],
                                     Bt[:, hd * 128:(hd + 1) * 128],
                                     X1g[:, j * 64:(j + 1) * 64],
                                     start=True, stop=True)
                Ytg = sc_p.tile([128, 8 * 65], BF16, tag=f"Yt{g8}", name=f"Yt{g8}")
                Yt3 = _r3(Ytg[:], 8)
                nc.vector.tensor_sub(Yt3[:, :, 0:64], _r3(X1g[:], 8),
                                     _r3(py[:], 8))
                nc.vector.memset(Yt3[:, :, 64:65], 1.0)
                Yts.append(Ytg)

            # ---- pOut (4-head groups; col 64 accumulates qr + Shat colsum = dn)
            for g in range(4):
                pout = psS_p.tile([128, 4 * 65], F32, tag="S", name="S")
                Ytg = Yts[g // 2]
                for j in range(4):
                    hd = g * 4 + j
                    jj = hd % 8
                    base = hd * KQW
                    if c == 0:
                        nc.tensor.matmul(pout[:, j * 65:(j + 1) * 65],
                                         Shat[:, hd * 128:(hd + 1) * 128],
                                         Ytg[:, jj * 65:(jj + 1) * 65],
                                         start=True, stop=True)
                    else:
                        for dc in range(2):
                            nc.tensor.matmul(
                                pout[:, j * 65:(j + 1) * 65],
                                KQ[dc][:, base + 129:base + 257],
                                t_Wb[:, hd * 130 + dc * 65:hd * 130 + (dc + 1) * 65],
                                start=(dc == 0), stop=False)
                        nc.tensor.matmul(pout[:, j * 65:(j + 1) * 65],
                                         Shat[:, hd * 128:(hd + 1) * 128],
                                         Ytg[:, jj * 65:(jj + 1) * 65],
                                         start=False, stop=True)
                pout3 = _r3(pout[:], 4)
                dng = ftmp_p.tile([128, 4], F32, tag="dng", name="dng")
                nc.vector.tensor_scalar_add(dng[:].unsqueeze(-1),
                                            pout3[:, :, 64:65], EPS)
                dnrg = ftmp_p.tile([128, 4], F32, tag="dnrg", name="dnrg")
                nc.vector.reciprocal(dnrg[:], dng[:])
                dnsg = ftmp_p.tile([128, 4], F32, tag="dnsg", name="dnsg")
                nc.vector.tensor_scalar_mul(dnsg[:], dnrg[:], SCALE)
                nc.vector.tensor_mul(
                    _r3(outc[:], NH)[:, g * 4:(g + 1) * 4, :],
                    pout3[:, :, 0:64], _b3(dnsg[:], 4, 64))

            # ---- W update (groups of 4 heads)
            for g in range(4):
                pw = psS_p.tile([128, 512], F32, tag="S", name="S")
                for j in range(4):
                    hd = g * 4 + j
                    for dc in range(2):
                        nc.tensor.matmul(
                            pw[:, j * 128 + dc * 64:j * 128 + (dc + 1) * 64],
                            fkn[:, dc * NH * 128 + hd * 128:
                                dc * NH * 128 + (hd + 1) * 128],
                            Yts[hd // 8][:, (hd % 8) * 65:(hd % 8) * 65 + 64],
                            start=True, stop=True)
                nc.vector.tensor_add(t_Wm[:, g * 512:(g + 1) * 512], pw[:],
                                     t_Wm[:, g * 512:(g + 1) * 512])
            Wb3 = t_Wb[:].rearrange("p (h o) -> p h o", h=NH)
            Wm3 = _r3(t_Wm[:], NH)
            for dc in range(2):
                nc.vector.tensor_copy(Wb3[:, :, dc * 65:dc * 65 + 64],
                                      Wm3[:, :, dc * 64:(dc + 1) * 64])
            # ---- r update (also lands in Wb cols 64/129 for the dn fold)
            for dc in range(2):
                rs = ftmp_p.tile([128, NH], F32, tag=f"rs{dc}", name=f"rs{dc}")
                nc.vector.tensor_reduce(rs[:], _r3(KQ[dc][:], NH)[:, :, 0:128],
                                        AX.X, OP.add)
                nc.gpsimd.tensor_add(t_r[dc][:], t_r[dc][:], rs[:])
                nc.vector.tensor_copy(Wb3[:, :, dc * 65 + 64:dc * 65 + 65],
                                      t_r[dc][:].unsqueeze(-1))

            # ================= outT + out-proj + residual + LN ============
            outT = oc_p.tile([128, NH * 64], BF16, tag="outT", name="outT")
            nc.scalar.dma_start_transpose(_r3(outT[:], 8), outc[:])
            hr = xln_p.tile([128, DM], F32, tag="hr", name="hr")
            nc.sync.dma_start(hr[:], d_hres[cs, :])
            x = xln_p.tile([128, DM], F32, tag="x", name="x")
            for og in range(2):
                pat = psS_p.tile([128, 512], F32, tag="S", name="S")
                for ic in range(8):
                    nc.tensor.matmul(pat[:], outT[:, ic * 128:(ic + 1) * 128],
                                     t_wo[ic][:, og * 512:(og + 1) * 512],
                                     start=(ic == 0), stop=(ic == 7))
                nc.vector.tensor_add(x[:, og * 512:(og + 1) * 512], pat[:],
                                     hr[:, og * 512:(og + 1) * 512])
            xsum = ftmp_p.tile([128, 1], F32, tag="xsum", name="xsum")
            nc.vector.tensor_reduce(xsum[:], x[:], AX.X, OP.add)
            nmu = ftmp_p.tile([128, 1], F32, tag="nmu", name="nmu")
            nc.vector.tensor_scalar_mul(nmu[:], xsum[:], -1.0 / DM)
            nc.gpsimd.tensor_scalar_add(x[:], x[:], nmu[:])
            vscr = hr[:].bitcast(BF16)[:, 0:DM]
            var = ftmp_p.tile([128, 1], F32, tag="var", name="var")
            nc.vector.scalar_tensor_tensor(vscr, x[:], 1.0, x[:],
                                           OP.mult, OP.mult, accum_out=var[:])
            vare = ftmp_p.tile([128, 1], F32, tag="vare", name="vare")
            nc.vector.tensor_scalar(vare[:], var[:], 1.0 / DM, float(LN_EPS),
                                    OP.mult, OP.add)
            sd = ftmp_p.tile([128, 1], F32, tag="sd", name="sd")
            nc.scalar.sqrt(sd[:], vare[:])
            rstd = ftmp_p.tile([128, 1], F32, tag="rstd", name="rstd")
            nc.vector.reciprocal(rstd[:], sd[:])
            nc.vector.scalar_tensor_tensor(x[:], x[:], rstd[:], t_lng[:],
                                           OP.mult, OP.mult)
            nc.gpsimd.tensor_add(x[:], x[:], t_lnb[:])
            nc.sync.dma_start(d_out[cs, :], x[:])

    return nc


# ---------------------------------------------------------------- host side
def _prep_core_inputs(h_b, W_qkvb, W_o, ln_g, ln_b):
    bf16 = ml_dtypes.bfloat16
    fp8 = ml_dtypes.float8_e4m3
    hT = np.ascontiguousarray(h_b.T).astype(fp8)                   # [1024, 2048]
    wq = np.zeros((DM, OTOT), dtype=np.float32)
    Wr = W_qkvb.reshape(NH, 193, DM)
    for hd in range(NH):
        wq[:, hd * 192:hd * 192 + 192] = Wr[hd, 0:192, :].T
        wq[:, OQKV + hd] = Wr[hd, 192, :]
    wq = (wq * WSC).astype(fp8)
    woT = np.ascontiguousarray(W_o.T).astype(bf16)                 # [i, o]
    lng = np.broadcast_to(ln_g[None, :], (128, DM)).astype(bf16).copy()
    lnb = np.broadcast_to(ln_b[None, :], (128, DM)).astype(bf16).copy()
    ii, jj = np.indices((128, 132))
    mSL = (jj < ii).astype(np.float32);  mSL[:, 128] = 1.0
    mLI = (jj <= ii).astype(np.float32); mLI[:, 128] = 1.0
    mUI = ((jj >= ii) & (jj < 128)).astype(np.float32)
    return {"hT": hT, "hres": np.ascontiguousarray(h_b, np.float32),
            "wqkv": wq, "woT": woT, "lng": lng, "lnb": lnb,
            "maskSL": mSL, "maskUI": mUI}


_cached = {}


def kernel(h, W_qkvb, W_o, ln_g, ln_b):
    h = np.asarray(h, np.float32)
    W_qkvb = np.asarray(W_qkvb, np.float32)
    W_o = np.asarray(W_o, np.float32)
    ln_g = np.asarray(ln_g, np.float32)
    ln_b = np.asarray(ln_b, np.float32)
    if "nc" not in _cached:
        _cached["nc"] = build_program()
    nc = _cached["nc"]
    in_maps = [_prep_core_inputs(h[:, b, :], W_qkvb, W_o, ln_g, ln_b)
               for b in range(BSZ)]
    res = run_bass_kernel_spmd(nc, in_maps, list(range(BSZ)),
                               trace=os.environ.get("BASS_TRACE", "") == "1")
    out = np.stack([res.results[b]["out"] for b in range(BSZ)], axis=1)
    kernel.last_exec_time_ns = res.exec_time_ns
    return out.astype(np.float32)


# revision 3
# speedup vs baseline: 1.1509x; 1.0035x over previous
"""Trainium2 Bass kernel v2: head-batched CudaNorm FastWeight DPFP layer.

Batch sharded across 8 cores (1 batch element per core). Per chunk (C=128):
qkvb projection (PSUM-wide groups), DPFP features batched over all 16 heads
via strided 3D APs, merged Gram matmuls ([A|kr|S1] + [S2|qr] per head in one
PSUM bank), batched mask/reduce extraction, 3-matmul Neumann solve per head
(Y = (I-N)(I+N^2)X0 via t1=NX0, t2=N*t1, pY=N*X1), transpose-free Shat,
single all-heads W state with one Pool copy per chunk.
"""
import os
import numpy as np
import ml_dtypes

import concourse.bass as bass
import concourse.mybir as mybir
from concourse.bass_utils import run_bass_kernel_spmd
from concourse.tile import TileContext
from concourse.vector_clock import ScopedClock, VectorClock
from contextlib import ExitStack

F32 = mybir.dt.float32
BF16 = mybir.dt.bfloat16
FP8 = mybir.dt.float8e4
WSC = 32.0
AF = mybir.ActivationFunctionType
OP = mybir.AluOpType
AX = mybir.AxisListType

SLEN, BSZ, DM = 2048, 8, 1024
NH, DH, NROLL = 16, 64, 2
D = 2 * NROLL * DH            # 256 feature dim (2 dc of 128)
C = 128                       # chunk length
NCH = SLEN // C               # 16 chunks
EPS, LN_EPS = 1e-5, 1e-5
SCALE = 1.0 / float(np.sqrt(DH))
OQKV = NH * 192               # 3072
OTOT = OQKV + NH              # 3088
KQW = 260                     # per-head stride in KQ tiles: K(128)|r(1)|Q(128)|pad

MAXW = 2


class PatchedTileContext(TileContext):
    """Work around walrus TPB sync-command limits: each instruction carries at
    most 2 sync commands (waits+updates); hoist excess waits onto preceding
    same-engine NoOps (1 wait each), and emit the kernel-tail drain's waits
    one-per-nop on SP."""

    def _lower_ordered_insts(self, ordered):
        for bb_name in list(ordered.keys()):
            new = []
            for inst in ordered[bb_name]:
                si = inst.sync_info
                nupd = len(si.on_update) if si is not None and si.on_update else 0
                maxw = max(0, MAXW - nupd)
                if si is not None and si.on_wait and len(si.on_wait) > maxw:
                    waits = list(si.on_wait)
                    excess = waits if maxw == 0 else waits[:-maxw]
                    keep = [] if maxw == 0 else waits[-maxw:]
                    for w in excess:
                        nop = mybir.InstNoOp(
                            name=self.nc.get_next_instruction_name(),
                            engine=inst.engine, ins=[], outs=[])
                        nop.sync_info = mybir.SyncInfo(on_wait=[w], on_update=[])
                        new.append(nop)
                    inst.sync_info = mybir.SyncInfo(
                        on_wait=keep, on_update=list(si.on_update or []))
                new.append(inst)
            ordered[bb_name] = new
        return super()._lower_ordered_insts(ordered)

    def _drain_and_barrier(self, tick_clock, wait_clock):
        gc = tick_clock.global_clock
        n = len(gc)
        for p in range(n):
            if gc[p] > 0:
                vc = VectorClock([gc[i] if i == p else 0 for i in range(n)])
                nop = self.nc.sync.nop(nofuse=True)
                wait_clock.add_sem_waits(nop.ins, ScopedClock({None: vc}))
        self.nc.sync.drain()
        self.nc.all_engine_barrier()
        assert self.sems is not None
        popped = self.nc._tile_sem_poison_stack.pop()
        assert popped is self._sem_poison
        self.nc.clear_and_free_semaphores(list(self.sems.allocated().values()))
        self.nc.all_engine_barrier()


def _r3(ap, h):
    return ap.rearrange("p (h o) -> p h o", h=h)


def _b3(ap, n, w):
    # [128, n] -> [128, n, w] with stride-0 inner axis
    return ap.unsqueeze(-1).broadcast_to([128, n, w])


# ---------------------------------------------------------------- program
def build_program(n_chunks=NCH):
    nc = bass.Bass()
    d_hT = nc.declare_dram_parameter("hT", [DM, SLEN], FP8, isOutput=False)
    d_hres = nc.declare_dram_parameter("hres", [SLEN, DM], F32, isOutput=False)
    d_w = nc.declare_dram_parameter("wqkv", [DM, OTOT], FP8, isOutput=False)
    d_wo = nc.declare_dram_parameter("woT", [DM, DM], BF16, isOutput=False)
    d_lng = nc.declare_dram_parameter("lng", [128, DM], BF16, isOutput=False)
    d_lnb = nc.declare_dram_parameter("lnb", [128, DM], BF16, isOutput=False)
    d_mSL = nc.declare_dram_parameter("maskSL", [128, 132], F32, isOutput=False)
    d_mUI = nc.declare_dram_parameter("maskUI", [128, 132], F32, isOutput=False)
    d_out = nc.declare_dram_parameter("out", [SLEN, DM], F32, isOutput=True)

    with PatchedTileContext(nc) as tc, ExitStack() as ctx:
        P = lambda name, bufs, **kw: ctx.enter_context(
            tc.tile_pool(name=name, bufs=bufs, **kw))
        const = P("const", 1)
        state = P("state", 1)
        hts_p = P("hts", 1)
        raw_p = P("raw", 1)
        sig_p = P("sig", 1)
        fx_p = P("fx", 1)      # one [128,2048] bf16 tag, rotated q/k
        ff_p = P("ff", 1)      # one [128,2048] f32 tag, rotated 4x (q/k halves)
        fn_p = P("fn", 2)
        kq_p = P("kq", 2)
        scr_p = P("scr", 1)
        cols_p = P("cols", 1)
        sc_p = P("sc", 1)
        oc_p = P("oc", 1)
        ftmp_p = P("ftmp", 4)
        xln_p = P("xln", 1)
        psG_p = P("psG", 1, space="PSUM")   # 4 banks: gram groups / projection
        psS_p = P("psS", 4, space="PSUM")   # 4 banks: solve/outproj/pW

        # ---- constants
        t_mSL = const.tile([128, 132], F32, tag="mSL", name="mSL"); nc.sync.dma_start(t_mSL[:], d_mSL[:])
        t_mUI = const.tile([128, 132], F32, tag="mUI", name="mUI"); nc.sync.dma_start(t_mUI[:], d_mUI[:])
        t_lng = const.tile([128, DM], BF16, tag="lng", name="lng"); nc.sync.dma_start(t_lng[:], d_lng[:])
        t_lnb = const.tile([128, DM], BF16, tag="lnb", name="lnb"); nc.sync.dma_start(t_lnb[:], d_lnb[:])
        tw_all = const.tile([128, 8 * OTOT], FP8, tag="tw", name="tw")
        nc.sync.dma_start(
            _r3(tw_all[:], 8),
            d_w[:].rearrange("(mc p) o -> mc p o", mc=8).transpose([1, 0, 2]))
        tw4 = _r3(tw_all[:], 8)
        t_wo = []
        for ic in range(8):
            t = const.tile([128, DM], BF16, tag=f"wo{ic}", name=f"wo{ic}")
            nc.sync.dma_start(t[:], d_wo[ic * 128:(ic + 1) * 128, :])
            t_wo.append(t)

        # ---- state (Wb per head: [Wdc0(64) | r0(1) | Wdc1(64) | r1(1)])
        t_Wm = state.tile([128, NH * 128], F32, tag="Wm", name="Wm")
        nc.vector.memset(t_Wm[:], 0.0)
        t_Wb = state.tile([128, NH * 130], BF16, tag="Wb", name="Wb")
        nc.vector.memset(t_Wb[:], 0.0)
        t_r = []
        for dc in range(2):
            r = state.tile([128, NH], F32, tag=f"r{dc}", name=f"r{dc}")
            nc.vector.memset(r[:], 0.0)
            t_r.append(r)

        for c in range(n_chunks):
            cs = slice((c % NCH) * 128, (c % NCH) * 128 + 128)
            # ================= projection =================================
            hts = hts_p.tile([128, 1024], FP8, tag="hts", name="hts")
            src = d_hT[:, cs].rearrange("(mc p) t -> mc p t", mc=8).transpose([1, 0, 2])
            nc.sync.dma_start(_r3(hts[:], 8), src)
            raw = raw_p.tile([128, OTOT], BF16, tag="raw", name="raw")
            pg = psG_p.tile([128, 2048], F32, tag="G", name="G")
            for g in range(4):
                for m2 in range(4):
                    nc.tensor.matmul(
                        pg[:, g * 512:(g + 1) * 512],
                        hts[:, m2 * 256:(m2 + 1) * 256].rearrange(
                            "p (two f) -> p two f", two=2),
                        tw4[:, 2 * m2:2 * m2 + 2, g * 512:(g + 1) * 512],
                        start=(m2 == 0), stop=(m2 == 3),
                        perf_mode=mybir.MatmulPerfMode.DoubleRow)
            nc.vector.tensor_scalar_mul(raw[:, 0:2048], pg[:], 1.0 / WSC)
            pg2 = psG_p.tile([128, 2048], F32, tag="G", name="G")
            for gi, (o0, ow) in enumerate([(2048, 512), (2560, 512), (OQKV, NH)]):
                for m2 in range(4):
                    nc.tensor.matmul(
                        pg2[:, gi * 512:gi * 512 + ow],
                        hts[:, m2 * 256:(m2 + 1) * 256].rearrange(
                            "p (two f) -> p two f", two=2),
                        tw4[:, 2 * m2:2 * m2 + 2, o0:o0 + ow],
                        start=(m2 == 0), stop=(m2 == 3),
                        perf_mode=mybir.MatmulPerfMode.DoubleRow)
            nc.vector.tensor_scalar_mul(
                _r3(raw[:, 2048:OTOT].rearrange("p (a b) -> p a b", a=1)
                    .squeeze(1), 1),
                pg2[:, 0:1040].rearrange("p (a b) -> p a b", a=1).squeeze(1),
                1.0 / WSC)
            sig = sig_p.tile([128, NH], F32, tag="sig", name="sig")
            nc.scalar.activation(sig[:], raw[:, OQKV:OQKV + NH], AF.Sigmoid)

            # ================= features (all heads, strided) ==============
            # fqn/fkn are dc-major: [128, (2 dc, 16 h, 128)] so each dc block
            # is contiguous and transposes in ONE batched DMA transpose.
            rawq = _r3(raw[:, 0:OQKV], NH)  # [128, 16, 192]
            fqn = fn_p.tile([128, NH * 256], BF16, tag="fqn", name="fqn")
            fkn = fn_p.tile([128, NH * 256], BF16, tag="fkn", name="fkn")
            for which, (coff, fdst) in enumerate(((0, fqn), (64, fkn))):
                eng = nc.vector
                xp = fx_p.tile([128, NH * 128], BF16, tag="xp", name="xp")
                x3 = _r3(xp[:], NH)
                nc.scalar.activation(x3[:, :, 0:64], rawq[:, :, coff:coff + 64],
                                     AF.Relu)
                nc.scalar.activation(x3[:, :, 64:128],
                                     rawq[:, :, coff:coff + 64], AF.Relu,
                                     scale=-1.0)
                sums = ftmp_p.tile([128, NH], F32, tag=f"sums{which}",
                                   name=f"sums{which}")
                for half in range(2):
                    hh = slice(half * 8, (half + 1) * 8)
                    f = ff_p.tile([128, 8 * 256], BF16, tag="f", name="f")
                    # f dc-major: [128, (2 dc, 8 h, 128)]
                    f4 = f[:].rearrange("p (d h o) -> p d h o", d=2, h=8)
                    xh = x3[:, hh, :]
                    eng.tensor_mul(f4[:, 0, :, 1:128], xh[:, :, 1:128],
                                   xh[:, :, 0:127])
                    eng.tensor_mul(f4[:, 0, :, 0:1], xh[:, :, 0:1],
                                   xh[:, :, 127:128])
                    eng.tensor_mul(f4[:, 1, :, 2:128], xh[:, :, 2:128],
                                   xh[:, :, 0:126])
                    eng.tensor_mul(f4[:, 1, :, 0:2], xh[:, :, 0:2],
                                   xh[:, :, 126:128])
                    sr = f[:].rearrange("p (d h o) -> p h d o", d=2, h=8)
                    nc.vector.tensor_reduce(sums[:, hh], sr, AX.XY, OP.add)
                    rec = ftmp_p.tile([128, 8], F32, tag=f"rec{which}{half}",
                                      name=f"rec{which}{half}")
                    nc.vector.reciprocal(rec[:], sums[:, hh])
                    fd4 = fdst[:].rearrange("p (d h o) -> p d h o", d=2, h=NH)
                    recb = rec[:].unsqueeze(1).unsqueeze(-1).broadcast_to(
                        [128, 2, 8, 128])
                    eng.tensor_mul(fd4[:, :, hh, :], f4, recb)

            # ---- transposes into KQ tiles: per head [K(128)|r(1)|Q(128)|pad]
            KQ = [kq_p.tile([128, NH * KQW], BF16, tag=f"KQ{dc}", name=f"KQ{dc}")
                  for dc in range(2)]
            for dc in range(2):
                for which, fsrc, off in ((0, fkn, 0), (1, fqn, 129)):
                    stage = ff_p.tile([128, 2048], BF16, tag="stg", name="stg")
                    eng = nc.sync if (dc + which) % 2 == 0 else nc.scalar
                    eng.dma_start_transpose(
                        _r3(stage[:], NH),
                        fsrc[:, dc * NH * 128:(dc + 1) * NH * 128])
                    nc.scalar.copy(_r3(KQ[dc][:], NH)[:, :, off:off + 128],
                                   _r3(stage[:], NH))
                nc.vector.tensor_copy(
                    _r3(KQ[dc][:], NH)[:, :, 128:129],
                    t_r[dc][:].unsqueeze(-1))

            # ================= gram + extraction ==========================
            kd_all = cols_p.tile([128, NH], F32, tag="kd", name="kd")
            tmpA = scr_p.tile([128, NH * 129], BF16, tag="tmpA", name="tmpA")
            Shat = scr_p.tile([128, NH * 128], BF16, tag="Shat", name="Shat")
            for g in range(4):
                pgr = psG_p.tile([128, 2048], F32, tag="G", name="G")
                for j in range(4):
                    hd = g * 4 + j
                    base = hd * KQW
                    for dc in range(2):
                        nc.tensor.matmul(pgr[:, j * 512:j * 512 + 257],
                                         KQ[dc][:, base:base + 128],
                                         KQ[dc][:, base:base + 257],
                                         start=(dc == 0), stop=(dc == 1))
                pgr3 = _r3(pgr[:], 4)
                tmpA3 = _r3(tmpA[:], NH)[:, g * 4:(g + 1) * 4, :]
                nc.vector.tensor_mul(
                    tmpA3, pgr3[:, :, 0:129],
                    t_mSL[:, 0:129].unsqueeze(1).broadcast_to([128, 4, 129]))
                nc.vector.tensor_reduce(kd_all[:, g * 4:(g + 1) * 4], tmpA3,
                                        AX.X, OP.add)
                nc.vector.tensor_mul(
                    _r3(Shat[:], NH)[:, g * 4:(g + 1) * 4, :],
                    pgr3[:, :, 129:257],
                    t_mUI[:, 0:128].unsqueeze(1).broadcast_to([128, 4, 128]))
            if c == 0:
                nc.vector.memset(kd_all[0:1, :], 1.0)

            # ================= chunk columns ==============================
            ceps = cols_p.tile([128, NH], F32, tag="ceps", name="ceps")
            nc.vector.tensor_scalar_add(ceps[:], kd_all[:], EPS)
            c_all = cols_p.tile([128, NH], F32, tag="c", name="c")
            nc.vector.reciprocal(c_all[:], ceps[:])
            t0 = cols_p.tile([128, NH], F32, tag="t0", name="t0")
            nc.vector.tensor_mul(t0[:], kd_all[:], c_all[:])
            cb_all = cols_p.tile([128, NH], F32, tag="cb", name="cb")
            nc.vector.tensor_mul(cb_all[:], t0[:], sig[:])
            cbc = cols_p.tile([128, NH], F32, tag="cbc", name="cbc")
            nc.vector.tensor_mul(cbc[:], cb_all[:], c_all[:])

            # ---- Nt (all heads) + Bt transposes
            Nt = scr_p.tile([128, NH * 128], BF16, tag="Nt", name="Nt")
            nc.vector.tensor_mul(_r3(Nt[:], NH),
                                 _r3(tmpA[:], NH)[:, :, 0:128],
                                 _b3(cbc[:], NH, 128))
            Bt = scr_p.tile([128, NH * 128], BF16, tag="Bt", name="Bt")
            nc.sync.dma_start_transpose(_r3(Bt[:], NH), Nt[:])

            # ================= solve + outputs + W ========================
            outc = oc_p.tile([128, NH * 64], BF16, tag="outc", name="outc")
            vall = oc_p.tile([128, NH * 64], BF16, tag="vall", name="vall")
            nc.scalar.copy(_r3(vall[:], NH), rawq[:, :, 128:192])
            Yts = []
            for g8 in range(2):
                hs = slice(g8 * 8, (g8 + 1) * 8)
                X0g = sc_p.tile([128, 512], BF16, tag=f"X0{g8}", name=f"X0{g8}")
                v3 = _r3(vall[:], NH)[:, hs, :]
                if c == 0:
                    nc.vector.tensor_mul(_r3(X0g[:], 8), v3,
                                         _b3(cb_all[:, hs], 8, 64))
                else:
                    pkw = psS_p.tile([128, 512], F32, tag="S", name="S")
                    for j in range(8):
                        hd = g8 * 8 + j
                        for dc in range(2):
                            nc.tensor.matmul(
                                pkw[:, j * 64:(j + 1) * 64],
                                KQ[dc][:, hd * KQW:hd * KQW + 128],
                                t_Wb[:, hd * 130 + dc * 65:hd * 130 + dc * 65 + 64],
                                start=(dc == 0), stop=(dc == 1))
                    xf = sc_p.tile([128, 512], F32, tag="xf", name="xf")
                    nc.vector.tensor_mul(_r3(xf[:], 8), _r3(pkw[:], 8),
                                         _b3(cbc[:, hs], 8, 64))
                    xf2 = sc_p.tile([128, 512], F32, tag="xf2", name="xf2")
                    nc.vector.tensor_mul(_r3(xf2[:], 8), v3,
                                         _b3(cb_all[:, hs], 8, 64))
                    nc.vector.tensor_sub(X0g[:], xf2[:], xf[:])
                pt1 = psS_p.tile([128, 512], F32, tag="S", name="S")
                for j in range(8):
                    hd = g8 * 8 + j
                    nc.tensor.matmul(pt1[:, j * 64:(j + 1) * 64],
                                     Bt[:, hd * 128:(hd + 1) * 128],
                                     X0g[:, j * 64:(j + 1) * 64],
                                     start=True, stop=True)
                t1s = sc_p.tile([128, 512], BF16, tag=f"t1s{g8}", name=f"t1s{g8}")
                nc.vector.tensor_copy(t1s[:], pt1[:])
                pt2 = psS_p.tile([128, 512], F32, tag="S", name="S")
                for j in range(8):
                    hd = g8 * 8 + j
                    nc.tensor.matmul(pt2[:, j * 64:(j + 1) * 64],
                                     Bt[:, hd * 128:(hd + 1) * 128],
                                     t1s[:, j * 64:(j + 1) * 64],
                                     start=True, stop=True)
                X1g = sc_p.tile([128, 512], BF16, tag=f"X1{g8}", name=f"X1{g8}")
                nc.vector.tensor_add(X1g[:], pt2[:], X0g[:])
                py = psS_p.tile([128, 512], F32, tag="S", name="S")
                for j in range(8):
                    hd = g8 * 8 + j
                    nc.tensor.matmul(py[:, j * 64:(j + 1) * 64],
                                     Bt[:, hd * 128:(hd + 1) * 128],
                                     X1g[:, j * 64:(j + 1) * 64],
                                     start=True, stop=True)
                Ytg = sc_p.tile([128, 8 * 65], BF16, tag=f"Yt{g8}", name=f"Yt{g8}")
                Yt3 = _r3(Ytg[:], 8)
                nc.vector.tensor_sub(Yt3[:, :, 0:64], _r3(X1g[:], 8),
                                     _r3(py[:], 8))
                nc.vector.memset(Yt3[:, :, 64:65], 1.0)
                Yts.append(Ytg)

            # ---- pOut (4-head groups; col 64 accumulates qr + Shat colsum = dn)
            for g in range(4):
                pout = psS_p.tile([128, 4 * 65], F32, tag="S", name="S")
                Ytg = Yts[g // 2]
                for j in range(4):
                    hd = g * 4 + j
                    jj = hd % 8
                    base = hd * KQW
                    if c == 0:
                        nc.tensor.matmul(pout[:, j * 65:(j + 1) * 65],
                                         Shat[:, hd * 128:(hd + 1) * 128],
                                         Ytg[:, jj * 65:(jj + 1) * 65],
                                         start=True, stop=True)
                    else:
                        for dc in range(2):
                            nc.tensor.matmul(
                                pout[:, j * 65:(j + 1) * 65],
                                KQ[dc][:, base + 129:base + 257],
                                t_Wb[:, hd * 130 + dc * 65:hd * 130 + (dc + 1) * 65],
                                start=(dc == 0), stop=False)
                        nc.tensor.matmul(pout[:, j * 65:(j + 1) * 65],
                                         Shat[:, hd * 128:(hd + 1) * 128],
                                         Ytg[:, jj * 65:(jj + 1) * 65],
                                         start=False, stop=True)
                pout3 = _r3(pout[:], 4)
                dng = ftmp_p.tile([128, 4], F32, tag="dng", name="dng")
                nc.vector.tensor_scalar_add(dng[:].unsqueeze(-1),
                                            pout3[:, :, 64:65], EPS)
                dnrg = ftmp_p.tile([128, 4], F32, tag="dnrg", name="dnrg")
                nc.vector.reciprocal(dnrg[:], dng[:])
                nc.vector.tensor_mul(
                    _r3(outc[:], NH)[:, g * 4:(g + 1) * 4, :],
                    pout3[:, :, 0:64], _b3(dnrg[:], 4, 64))

            # ---- W update (groups of 4 heads)
            for g in range(4):
                pw = psS_p.tile([128, 512], F32, tag="S", name="S")
                for j in range(4):
                    hd = g * 4 + j
                    for dc in range(2):
                        nc.tensor.matmul(
                            pw[:, j * 128 + dc * 64:j * 128 + (dc + 1) * 64],
                            fkn[:, dc * NH * 128 + hd * 128:
                                dc * NH * 128 + (hd + 1) * 128],
                            Yts[hd // 8][:, (hd % 8) * 65:(hd % 8) * 65 + 64],
                            start=True, stop=True)
                nc.vector.tensor_add(t_Wm[:, g * 512:(g + 1) * 512], pw[:],
                                     t_Wm[:, g * 512:(g + 1) * 512])
            Wb3 = t_Wb[:].rearrange("p (h o) -> p h o", h=NH)
            Wm3 = _r3(t_Wm[:], NH)
            for dc in range(2):
                nc.vector.tensor_copy(Wb3[:, :, dc * 65:dc * 65 + 64],
                                      Wm3[:, :, dc * 64:(dc + 1) * 64])
            # ---- r update (also lands in Wb cols 64/129 for the dn fold)
            for dc in range(2):
                rs = ftmp_p.tile([128, NH], F32, tag=f"rs{dc}", name=f"rs{dc}")
                nc.vector.tensor_reduce(rs[:], _r3(KQ[dc][:], NH)[:, :, 0:128],
                                        AX.X, OP.add)
                nc.gpsimd.tensor_add(t_r[dc][:], t_r[dc][:], rs[:])
                nc.vector.tensor_copy(Wb3[:, :, dc * 65 + 64:dc * 65 + 65],
                                      t_r[dc][:].unsqueeze(-1))

            # ================= outT + out-proj + residual + LN ============
            outT = oc_p.tile([128, NH * 64], BF16, tag="outT", name="outT")
            nc.scalar.dma_start_transpose(_r3(outT[:], 8), outc[:])
            hr = xln_p.tile([128, DM], F32, tag="hr", name="hr")
            nc.sync.dma_start(hr[:], d_hres[cs, :])
            x = xln_p.tile([128, DM], F32, tag="x", name="x")
            for og in range(2):
                pat = psS_p.tile([128, 512], F32, tag="S", name="S")
                for ic in range(8):
                    nc.tensor.matmul(pat[:], outT[:, ic * 128:(ic + 1) * 128],
                                     t_wo[ic][:, og * 512:(og + 1) * 512],
                                     start=(ic == 0), stop=(ic == 7))
                nc.vector.tensor_add(x[:, og * 512:(og + 1) * 512], pat[:],
                                     hr[:, og * 512:(og + 1) * 512])
            xsum = ftmp_p.tile([128, 1], F32, tag="xsum", name="xsum")
            nc.vector.tensor_reduce(xsum[:], x[:], AX.X, OP.add)
            nmu = ftmp_p.tile([128, 1], F32, tag="nmu", name="nmu")
            nc.vector.tensor_scalar_mul(nmu[:], xsum[:], -1.0 / DM)
            nc.gpsimd.tensor_scalar_add(x[:], x[:], nmu[:])
            vscr = hr[:].bitcast(BF16)[:, 0:DM]
            var = ftmp_p.tile([128, 1], F32, tag="var", name="var")
            nc.vector.scalar_tensor_tensor(vscr, x[:], 1.0, x[:],
                                           OP.mult, OP.mult, accum_out=var[:])
            vare = ftmp_p.tile([128, 1], F32, tag="vare", name="vare")
            nc.vector.tensor_scalar(vare[:], var[:], 1.0 / DM, float(LN_EPS),
                                    OP.mult, OP.add)
            sd = ftmp_p.tile([128, 1], F32, tag="sd", name="sd")
            nc.scalar.sqrt(sd[:], vare[:])
            rstd = ftmp_p.tile([128, 1], F32, tag="rstd", name="rstd")
            nc.vector.reciprocal(rstd[:], sd[:])
            nc.vector.scalar_tensor_tensor(x[:], x[:], rstd[:], t_lng[:],
                                           OP.mult, OP.mult)
            nc.gpsimd.tensor_add(x[:], x[:], t_lnb[:])
            nc.sync.dma_start(d_out[cs, :], x[:])

    return nc


# ---------------------------------------------------------------- host side
def _prep_core_inputs(h_b, W_qkvb, W_o, ln_g, ln_b):
    bf16 = ml_dtypes.bfloat16
    fp8 = ml_dtypes.float8_e4m3
    hT = np.ascontiguousarray(h_b.T).astype(fp8)                   # [1024, 2048]
    wq = np.zeros((DM, OTOT), dtype=np.float32)
    Wr = W_qkvb.reshape(NH, 193, DM)
    for hd in range(NH):
        wq[:, hd * 192:hd * 192 + 192] = Wr[hd, 0:192, :].T
        wq[:, OQKV + hd] = Wr[hd, 192, :]
    wq = (wq * WSC).astype(fp8)
    woT = np.ascontiguousarray(W_o.T * SCALE).astype(bf16)         # [i, o]
    lng = np.broadcast_to(ln_g[None, :], (128, DM)).astype(bf16).copy()
    lnb = np.broadcast_to(ln_b[None, :], (128, DM)).astype(bf16).copy()
    ii, jj = np.indices((128, 132))
    mSL = (jj < ii).astype(np.float32);  mSL[:, 128] = 1.0
    mLI = (jj <= ii).astype(np.float32); mLI[:, 128] = 1.0
    mUI = ((jj >= ii) & (jj < 128)).astype(np.float32)
    return {"hT": hT, "hres": np.ascontiguousarray(h_b, np.float32),
            "wqkv": wq, "woT": woT, "lng": lng, "lnb": lnb,
            "maskSL": mSL, "maskUI": mUI}


_cached = {}


def kernel(h, W_qkvb, W_o, ln_g, ln_b):
    h = np.asarray(h, np.float32)
    W_qkvb = np.asarray(W_qkvb, np.float32)
    W_o = np.asarray(W_o, np.float32)
    ln_g = np.asarray(ln_g, np.float32)
    ln_b = np.asarray(ln_b, np.float32)
    if "nc" not in _cached:
        _cached["nc"] = build_program()
    nc = _cached["nc"]
    in_maps = [_prep_core_inputs(h[:, b, :], W_qkvb, W_o, ln_g, ln_b)
               for b in range(BSZ)]
    res = run_bass_kernel_spmd(nc, in_maps, list(range(BSZ)),
                               trace=os.environ.get("BASS_TRACE", "") == "1")
    out = np.stack([res.results[b]["out"] for b in range(BSZ)], axis=1)
    kernel.last_exec_time_ns = res.exec_time_ns
    return out.astype(np.float32)


# revision 4
# speedup vs baseline: 1.2099x; 1.0513x over previous
"""Trainium2 Bass kernel v2: head-batched CudaNorm FastWeight DPFP layer.

Batch sharded across 8 cores (1 batch element per core). Per chunk (C=128):
qkvb projection (PSUM-wide groups), DPFP features batched over all 16 heads
via strided 3D APs, merged Gram matmuls ([A|kr|S1] + [S2|qr] per head in one
PSUM bank), batched mask/reduce extraction, 3-matmul Neumann solve per head
(Y = (I-N)(I+N^2)X0 via t1=NX0, t2=N*t1, pY=N*X1), transpose-free Shat,
single all-heads W state with one Pool copy per chunk.
"""
import os
import numpy as np
import ml_dtypes

import concourse.bass as bass
import concourse.mybir as mybir
from concourse.bass_utils import run_bass_kernel_spmd
from concourse.tile import TileContext
from concourse.vector_clock import ScopedClock, VectorClock
from contextlib import ExitStack

F32 = mybir.dt.float32
BF16 = mybir.dt.bfloat16
FP8 = mybir.dt.float8e4
WSC = 32.0
AF = mybir.ActivationFunctionType
OP = mybir.AluOpType
AX = mybir.AxisListType

SLEN, BSZ, DM = 2048, 8, 1024
NH, DH, NROLL = 16, 64, 2
D = 2 * NROLL * DH            # 256 feature dim (2 dc of 128)
C = 128                       # chunk length
NCH = SLEN // C               # 16 chunks
EPS, LN_EPS = 1e-5, 1e-5
SCALE = 1.0 / float(np.sqrt(DH))
OQKV = NH * 192               # 3072
OTOT = OQKV + NH              # 3088
KQW = 260                     # per-head stride in KQ tiles: K(128)|r(1)|Q(128)|pad

MAXW = 2


class PatchedTileContext(TileContext):
    """Work around walrus TPB sync-command limits: each instruction carries at
    most 2 sync commands (waits+updates); hoist excess waits onto preceding
    same-engine NoOps (1 wait each), and emit the kernel-tail drain's waits
    one-per-nop on SP."""

    def _lower_ordered_insts(self, ordered):
        for bb_name in list(ordered.keys()):
            new = []
            for inst in ordered[bb_name]:
                si = inst.sync_info
                nupd = len(si.on_update) if si is not None and si.on_update else 0
                maxw = max(0, MAXW - nupd)
                if si is not None and si.on_wait and len(si.on_wait) > maxw:
                    waits = list(si.on_wait)
                    excess = waits if maxw == 0 else waits[:-maxw]
                    keep = [] if maxw == 0 else waits[-maxw:]
                    for w in excess:
                        nop = mybir.InstNoOp(
                            name=self.nc.get_next_instruction_name(),
                            engine=inst.engine, ins=[], outs=[])
                        nop.sync_info = mybir.SyncInfo(on_wait=[w], on_update=[])
                        new.append(nop)
                    inst.sync_info = mybir.SyncInfo(
                        on_wait=keep, on_update=list(si.on_update or []))
                new.append(inst)
            ordered[bb_name] = new
        return super()._lower_ordered_insts(ordered)

    def _drain_and_barrier(self, tick_clock, wait_clock):
        gc = tick_clock.global_clock
        n = len(gc)
        for p in range(n):
            if gc[p] > 0:
                vc = VectorClock([gc[i] if i == p else 0 for i in range(n)])
                nop = self.nc.sync.nop(nofuse=True)
                wait_clock.add_sem_waits(nop.ins, ScopedClock({None: vc}))
        self.nc.sync.drain()
        self.nc.all_engine_barrier()
        assert self.sems is not None
        popped = self.nc._tile_sem_poison_stack.pop()
        assert popped is self._sem_poison
        self.nc.clear_and_free_semaphores(list(self.sems.allocated().values()))
        self.nc.all_engine_barrier()


def _r3(ap, h):
    return ap.rearrange("p (h o) -> p h o", h=h)


def _b3(ap, n, w):
    # [128, n] -> [128, n, w] with stride-0 inner axis
    return ap.unsqueeze(-1).broadcast_to([128, n, w])


# ---------------------------------------------------------------- program
def build_program(n_chunks=NCH):
    nc = bass.Bass()
    d_hT = nc.declare_dram_parameter("hT", [DM, SLEN], FP8, isOutput=False)
    d_hres = nc.declare_dram_parameter("hres", [SLEN, DM], F32, isOutput=False)
    d_w = nc.declare_dram_parameter("wqkv", [DM, OTOT], FP8, isOutput=False)
    d_wo = nc.declare_dram_parameter("woT", [DM, DM], BF16, isOutput=False)
    d_lng = nc.declare_dram_parameter("lng", [128, DM], BF16, isOutput=False)
    d_lnb = nc.declare_dram_parameter("lnb", [128, DM], BF16, isOutput=False)
    d_mSL = nc.declare_dram_parameter("maskSL", [128, 132], F32, isOutput=False)
    d_mUI = nc.declare_dram_parameter("maskUI", [128, 132], F32, isOutput=False)
    d_out = nc.declare_dram_parameter("out", [SLEN, DM], F32, isOutput=True)

    with PatchedTileContext(nc) as tc, ExitStack() as ctx:
        P = lambda name, bufs, **kw: ctx.enter_context(
            tc.tile_pool(name=name, bufs=bufs, **kw))
        const = P("const", 1)
        state = P("state", 1)
        hts_p = P("hts", 1)
        raw_p = P("raw", 1)
        sig_p = P("sig", 1)
        fx_p = P("fx", 1)      # one [128,2048] bf16 tag, rotated q/k
        ff_p = P("ff", 1)      # one [128,2048] f32 tag, rotated 4x (q/k halves)
        fn_p = P("fn", 2)
        kq_p = P("kq", 2)
        scr_p = P("scr", 1)
        cols_p = P("cols", 1)
        sc_p = P("sc", 1)
        oc_p = P("oc", 1)
        ftmp_p = P("ftmp", 4)
        xln_p = P("xln", 1)
        psG_p = P("psG", 1, space="PSUM")   # 4 banks: gram groups / projection
        psS_p = P("psS", 4, space="PSUM")   # 4 banks: solve/outproj/pW

        # ---- constants
        t_mSL = const.tile([128, 132], F32, tag="mSL", name="mSL"); nc.sync.dma_start(t_mSL[:], d_mSL[:])
        t_mUI = const.tile([128, 132], F32, tag="mUI", name="mUI"); nc.sync.dma_start(t_mUI[:], d_mUI[:])
        t_lng = const.tile([128, DM], BF16, tag="lng", name="lng"); nc.sync.dma_start(t_lng[:], d_lng[:])
        t_lnb = const.tile([128, DM], BF16, tag="lnb", name="lnb"); nc.sync.dma_start(t_lnb[:], d_lnb[:])
        tw_all = const.tile([128, 8 * OTOT], FP8, tag="tw", name="tw")
        nc.sync.dma_start(
            _r3(tw_all[:], 8),
            d_w[:].rearrange("(mc p) o -> mc p o", mc=8).transpose([1, 0, 2]))
        tw4 = _r3(tw_all[:], 8)
        t_wo = []
        for ic in range(8):
            t = const.tile([128, DM], BF16, tag=f"wo{ic}", name=f"wo{ic}")
            nc.sync.dma_start(t[:], d_wo[ic * 128:(ic + 1) * 128, :])
            t_wo.append(t)

        # ---- state (Wb per head: [Wdc0(64) | r0(1) | Wdc1(64) | r1(1)])
        t_Wm = state.tile([128, NH * 128], F32, tag="Wm", name="Wm")
        nc.vector.memset(t_Wm[:], 0.0)
        t_Wb = state.tile([128, NH * 130], BF16, tag="Wb", name="Wb")
        nc.vector.memset(t_Wb[:], 0.0)
        t_r = []
        for dc in range(2):
            r = state.tile([128, NH], F32, tag=f"r{dc}", name=f"r{dc}")
            nc.vector.memset(r[:], 0.0)
            t_r.append(r)

        for c in range(n_chunks):
            cs = slice((c % NCH) * 128, (c % NCH) * 128 + 128)
            # ================= projection =================================
            hts = hts_p.tile([128, 1024], FP8, tag="hts", name="hts")
            src = d_hT[:, cs].rearrange("(mc p) t -> mc p t", mc=8).transpose([1, 0, 2])
            nc.sync.dma_start(_r3(hts[:], 8), src)
            raw = raw_p.tile([128, OTOT], BF16, tag="raw", name="raw")
            pg = psG_p.tile([128, 2048], F32, tag="G", name="G")
            for g in range(4):
                for m2 in range(4):
                    nc.tensor.matmul(
                        pg[:, g * 512:(g + 1) * 512],
                        hts[:, m2 * 256:(m2 + 1) * 256].rearrange(
                            "p (two f) -> p two f", two=2),
                        tw4[:, 2 * m2:2 * m2 + 2, g * 512:(g + 1) * 512],
                        start=(m2 == 0), stop=(m2 == 3),
                        perf_mode=mybir.MatmulPerfMode.DoubleRow)
            nc.vector.tensor_scalar_mul(raw[:, 0:2048], pg[:], 1.0 / WSC)
            pg2 = psG_p.tile([128, 2048], F32, tag="G", name="G")
            for gi, (o0, ow) in enumerate([(2048, 512), (2560, 512), (OQKV, NH)]):
                for m2 in range(4):
                    nc.tensor.matmul(
                        pg2[:, gi * 512:gi * 512 + ow],
                        hts[:, m2 * 256:(m2 + 1) * 256].rearrange(
                            "p (two f) -> p two f", two=2),
                        tw4[:, 2 * m2:2 * m2 + 2, o0:o0 + ow],
                        start=(m2 == 0), stop=(m2 == 3),
                        perf_mode=mybir.MatmulPerfMode.DoubleRow)
            nc.vector.tensor_scalar_mul(
                _r3(raw[:, 2048:OTOT].rearrange("p (a b) -> p a b", a=1)
                    .squeeze(1), 1),
                pg2[:, 0:1040].rearrange("p (a b) -> p a b", a=1).squeeze(1),
                1.0 / WSC)
            sig = sig_p.tile([128, NH], F32, tag="sig", name="sig")
            nc.scalar.activation(sig[:], raw[:, OQKV:OQKV + NH], AF.Sigmoid)

            # ================= features (all heads, strided) ==============
            # fqn/fkn are dc-major: [128, (2 dc, 16 h, 128)] so each dc block
            # is contiguous and transposes in ONE batched DMA transpose.
            rawq = _r3(raw[:, 0:OQKV], NH)  # [128, 16, 192]
            fqn = fn_p.tile([128, NH * 256], BF16, tag="fqn", name="fqn")
            fkn = fn_p.tile([128, NH * 256], BF16, tag="fkn", name="fkn")
            for which, (coff, fdst) in enumerate(((0, fqn), (64, fkn))):
                eng = nc.vector
                xp = fx_p.tile([128, NH * 128], BF16, tag="xp", name="xp")
                x3 = _r3(xp[:], NH)
                nc.scalar.activation(x3[:, :, 0:64], rawq[:, :, coff:coff + 64],
                                     AF.Relu)
                nc.scalar.activation(x3[:, :, 64:128],
                                     rawq[:, :, coff:coff + 64], AF.Relu,
                                     scale=-1.0)
                sums = ftmp_p.tile([128, NH], F32, tag=f"sums{which}",
                                   name=f"sums{which}")
                for half in range(2):
                    hh = slice(half * 8, (half + 1) * 8)
                    f = ff_p.tile([128, 8 * 256], BF16, tag="f", name="f")
                    # f dc-major: [128, (2 dc, 8 h, 128)]
                    f4 = f[:].rearrange("p (d h o) -> p d h o", d=2, h=8)
                    xh = x3[:, hh, :]
                    eng.tensor_mul(f4[:, 0, :, 1:128], xh[:, :, 1:128],
                                   xh[:, :, 0:127])
                    eng.tensor_mul(f4[:, 0, :, 0:1], xh[:, :, 0:1],
                                   xh[:, :, 127:128])
                    eng.tensor_mul(f4[:, 1, :, 2:128], xh[:, :, 2:128],
                                   xh[:, :, 0:126])
                    eng.tensor_mul(f4[:, 1, :, 0:2], xh[:, :, 0:2],
                                   xh[:, :, 126:128])
                    sr = f[:].rearrange("p (d h o) -> p h d o", d=2, h=8)
                    nc.vector.tensor_reduce(sums[:, hh], sr, AX.XY, OP.add)
                    rec = ftmp_p.tile([128, 8], F32, tag=f"rec{which}{half}",
                                      name=f"rec{which}{half}")
                    nc.vector.reciprocal(rec[:], sums[:, hh])
                    fd4 = fdst[:].rearrange("p (d h o) -> p d h o", d=2, h=NH)
                    recb = rec[:].unsqueeze(1).unsqueeze(-1).broadcast_to(
                        [128, 2, 8, 128])
                    eng.tensor_mul(fd4[:, :, hh, :], f4, recb)

            # ---- transposes into KQ tiles: per head [K(128)|r(1)|Q(128)|pad]
            KQ = [kq_p.tile([128, NH * KQW], BF16, tag=f"KQ{dc}", name=f"KQ{dc}")
                  for dc in range(2)]
            for dc in range(2):
                for which, fsrc, off in ((0, fkn, 0), (1, fqn, 129)):
                    stage = ff_p.tile([128, 2048], BF16, tag="stg", name="stg")
                    eng = nc.sync if (dc + which) % 2 == 0 else nc.scalar
                    eng.dma_start_transpose(
                        _r3(stage[:], NH),
                        fsrc[:, dc * NH * 128:(dc + 1) * NH * 128])
                    nc.scalar.copy(_r3(KQ[dc][:], NH)[:, :, off:off + 128],
                                   _r3(stage[:], NH))
                nc.vector.tensor_copy(
                    _r3(KQ[dc][:], NH)[:, :, 128:129],
                    t_r[dc][:].unsqueeze(-1))

            # ================= gram + extraction ==========================
            kd_all = cols_p.tile([128, NH], F32, tag="kd", name="kd")
            tmpA = scr_p.tile([128, NH * 129], BF16, tag="tmpA", name="tmpA")
            Shat = scr_p.tile([128, NH * 128], BF16, tag="Shat", name="Shat")
            for g in range(4):
                pgr = psG_p.tile([128, 2048], F32, tag="G", name="G")
                for j in range(4):
                    hd = g * 4 + j
                    base = hd * KQW
                    for dc in range(2):
                        nc.tensor.matmul(pgr[:, j * 512:j * 512 + 257],
                                         KQ[dc][:, base:base + 128],
                                         KQ[dc][:, base:base + 257],
                                         start=(dc == 0), stop=(dc == 1))
                pgr3 = _r3(pgr[:], 4)
                tmpA3 = _r3(tmpA[:], NH)[:, g * 4:(g + 1) * 4, :]
                nc.vector.tensor_mul(
                    tmpA3, pgr3[:, :, 0:129],
                    t_mSL[:, 0:129].unsqueeze(1).broadcast_to([128, 4, 129]))
                nc.vector.tensor_reduce(kd_all[:, g * 4:(g + 1) * 4], tmpA3,
                                        AX.X, OP.add)
                nc.vector.tensor_mul(
                    _r3(Shat[:], NH)[:, g * 4:(g + 1) * 4, :],
                    pgr3[:, :, 129:257],
                    t_mUI[:, 0:128].unsqueeze(1).broadcast_to([128, 4, 128]))
            if c == 0:
                nc.vector.memset(kd_all[0:1, :], 1.0)

            # ================= chunk columns ==============================
            ceps = cols_p.tile([128, NH], F32, tag="ceps", name="ceps")
            nc.vector.tensor_scalar_add(ceps[:], kd_all[:], EPS)
            c_all = cols_p.tile([128, NH], F32, tag="c", name="c")
            nc.vector.reciprocal(c_all[:], ceps[:])
            t0 = cols_p.tile([128, NH], F32, tag="t0", name="t0")
            nc.vector.tensor_mul(t0[:], kd_all[:], c_all[:])
            cb_all = cols_p.tile([128, NH], F32, tag="cb", name="cb")
            nc.vector.tensor_mul(cb_all[:], t0[:], sig[:])
            cbc = cols_p.tile([128, NH], F32, tag="cbc", name="cbc")
            nc.vector.tensor_mul(cbc[:], cb_all[:], c_all[:])

            # ---- Nt (all heads) + Bt transposes
            Nt = scr_p.tile([128, NH * 128], BF16, tag="Nt", name="Nt")
            nc.vector.tensor_mul(_r3(Nt[:], NH),
                                 _r3(tmpA[:], NH)[:, :, 0:128],
                                 _b3(cbc[:], NH, 128))
            Bt = scr_p.tile([128, NH * 128], BF16, tag="Bt", name="Bt")
            nc.sync.dma_start_transpose(_r3(Bt[:], NH), Nt[:])

            # ================= solve + outputs + W ========================
            outc = oc_p.tile([128, NH * 64], BF16, tag="outc", name="outc")
            vcb = oc_p.tile([128, NH * 64], BF16, tag="vall", name="vall")
            nc.vector.tensor_mul(_r3(vcb[:], NH), rawq[:, :, 128:192],
                                 _b3(cb_all[:], NH, 64))
            Yts = []
            for g8 in range(2):
                hs = slice(g8 * 8, (g8 + 1) * 8)
                if c > 0:
                    pkw = psS_p.tile([128, 512], F32, tag="S", name="S")
                    for j in range(8):
                        hd = g8 * 8 + j
                        for dc in range(2):
                            nc.tensor.matmul(
                                pkw[:, j * 64:(j + 1) * 64],
                                KQ[dc][:, hd * KQW:hd * KQW + 128],
                                t_Wb[:, hd * 130 + dc * 65:hd * 130 + dc * 65 + 64],
                                start=(dc == 0), stop=(dc == 1))
                    xf = sc_p.tile([128, 512], F32, tag="xf", name="xf")
                    nc.vector.tensor_mul(_r3(xf[:], 8), _r3(pkw[:], 8),
                                         _b3(cbc[:, hs], 8, 64))
                    X0g = sc_p.tile([128, 512], BF16, tag=f"X0{g8}",
                                    name=f"X0{g8}")
                    nc.vector.tensor_sub(
                        X0g[:], vcb[:, g8 * 512:(g8 + 1) * 512], xf[:])
                X0ap = (vcb[:, g8 * 512:(g8 + 1) * 512] if c == 0
                        else X0g[:])
                pt1 = psS_p.tile([128, 512], F32, tag="S", name="S")
                for j in range(8):
                    hd = g8 * 8 + j
                    nc.tensor.matmul(pt1[:, j * 64:(j + 1) * 64],
                                     Bt[:, hd * 128:(hd + 1) * 128],
                                     X0ap[:, j * 64:(j + 1) * 64],
                                     start=True, stop=True)
                t1s = sc_p.tile([128, 512], BF16, tag=f"t1s{g8}", name=f"t1s{g8}")
                nc.vector.tensor_copy(t1s[:], pt1[:])
                pt2 = psS_p.tile([128, 512], F32, tag="S", name="S")
                for j in range(8):
                    hd = g8 * 8 + j
                    nc.tensor.matmul(pt2[:, j * 64:(j + 1) * 64],
                                     Bt[:, hd * 128:(hd + 1) * 128],
                                     t1s[:, j * 64:(j + 1) * 64],
                                     start=True, stop=True)
                X1g = sc_p.tile([128, 512], BF16, tag=f"X1{g8}", name=f"X1{g8}")
                nc.vector.tensor_add(X1g[:], pt2[:], X0ap)
                py = psS_p.tile([128, 512], F32, tag="S", name="S")
                for j in range(8):
                    hd = g8 * 8 + j
                    nc.tensor.matmul(py[:, j * 64:(j + 1) * 64],
                                     Bt[:, hd * 128:(hd + 1) * 128],
                                     X1g[:, j * 64:(j + 1) * 64],
                                     start=True, stop=True)
                Ytg = sc_p.tile([128, 8 * 65], BF16, tag=f"Yt{g8}", name=f"Yt{g8}")
                Yt3 = _r3(Ytg[:], 8)
                nc.vector.tensor_sub(Yt3[:, :, 0:64], _r3(X1g[:], 8),
                                     _r3(py[:], 8))
                if c == 0:
                    nc.vector.memset(Yt3[:, :, 64:65], 1.0)
                Yts.append(Ytg)

            # ---- pOut (4-head groups; col 64 accumulates qr + Shat colsum = dn)
            for g in range(4):
                pout = psS_p.tile([128, 4 * 65], F32, tag="S", name="S")
                Ytg = Yts[g // 2]
                for j in range(4):
                    hd = g * 4 + j
                    jj = hd % 8
                    base = hd * KQW
                    if c == 0:
                        nc.tensor.matmul(pout[:, j * 65:(j + 1) * 65],
                                         Shat[:, hd * 128:(hd + 1) * 128],
                                         Ytg[:, jj * 65:(jj + 1) * 65],
                                         start=True, stop=True)
                    else:
                        for dc in range(2):
                            nc.tensor.matmul(
                                pout[:, j * 65:(j + 1) * 65],
                                KQ[dc][:, base + 129:base + 257],
                                t_Wb[:, hd * 130 + dc * 65:hd * 130 + (dc + 1) * 65],
                                start=(dc == 0), stop=False)
                        nc.tensor.matmul(pout[:, j * 65:(j + 1) * 65],
                                         Shat[:, hd * 128:(hd + 1) * 128],
                                         Ytg[:, jj * 65:(jj + 1) * 65],
                                         start=False, stop=True)
                pout3 = _r3(pout[:], 4)
                dnrg = ftmp_p.tile([128, 4], F32, tag="dnrg", name="dnrg")
                nc.vector.reciprocal(dnrg[:].unsqueeze(-1),
                                     pout3[:, :, 64:65])
                nc.vector.tensor_mul(
                    _r3(outc[:], NH)[:, g * 4:(g + 1) * 4, :],
                    pout3[:, :, 0:64], _b3(dnrg[:], 4, 64))

            # ---- W update (groups of 4 heads)
            for g in range(4):
                pw = psS_p.tile([128, 512], F32, tag="S", name="S")
                for j in range(4):
                    hd = g * 4 + j
                    for dc in range(2):
                        nc.tensor.matmul(
                            pw[:, j * 128 + dc * 64:j * 128 + (dc + 1) * 64],
                            fkn[:, dc * NH * 128 + hd * 128:
                                dc * NH * 128 + (hd + 1) * 128],
                            Yts[hd // 8][:, (hd % 8) * 65:(hd % 8) * 65 + 64],
                            start=True, stop=True)
                nc.vector.tensor_add(t_Wm[:, g * 512:(g + 1) * 512], pw[:],
                                     t_Wm[:, g * 512:(g + 1) * 512])
            Wb3 = t_Wb[:].rearrange("p (h o) -> p h o", h=NH)
            Wm3 = _r3(t_Wm[:], NH)
            for dc in range(2):
                nc.vector.tensor_copy(Wb3[:, :, dc * 65:dc * 65 + 64],
                                      Wm3[:, :, dc * 64:(dc + 1) * 64])
            # ---- r update (also lands in Wb cols 64/129 for the dn fold)
            for dc in range(2):
                rs = ftmp_p.tile([128, NH], F32, tag=f"rs{dc}", name=f"rs{dc}")
                nc.vector.tensor_reduce(rs[:], _r3(KQ[dc][:], NH)[:, :, 0:128],
                                        AX.X, OP.add)
                nc.gpsimd.tensor_add(t_r[dc][:], t_r[dc][:], rs[:])
                nc.vector.tensor_copy(Wb3[:, :, dc * 65 + 64:dc * 65 + 65],
                                      t_r[dc][:].unsqueeze(-1))

            # ================= outT + out-proj + residual + LN ============
            outT = oc_p.tile([128, NH * 64], BF16, tag="outT", name="outT")
            nc.scalar.dma_start_transpose(_r3(outT[:], 8), outc[:])
            hr = xln_p.tile([128, DM], F32, tag="hr", name="hr")
            nc.sync.dma_start(hr[:], d_hres[cs, :])
            x = xln_p.tile([128, DM], F32, tag="x", name="x")
            for og in range(2):
                pat = psS_p.tile([128, 512], F32, tag="S", name="S")
                for ic in range(8):
                    nc.tensor.matmul(pat[:], outT[:, ic * 128:(ic + 1) * 128],
                                     t_wo[ic][:, og * 512:(og + 1) * 512],
                                     start=(ic == 0), stop=(ic == 7))
                nc.vector.tensor_add(x[:, og * 512:(og + 1) * 512], pat[:],
                                     hr[:, og * 512:(og + 1) * 512])
            xsum = ftmp_p.tile([128, 1], F32, tag="xsum", name="xsum")
            nc.vector.tensor_reduce(xsum[:], x[:], AX.X, OP.add)
            nmu = ftmp_p.tile([128, 1], F32, tag="nmu", name="nmu")
            nc.vector.tensor_scalar_mul(nmu[:], xsum[:], -1.0 / DM)
            nc.gpsimd.tensor_scalar_add(x[:], x[:], nmu[:])
            vscr = hr[:].bitcast(BF16)[:, 0:DM]
            var = ftmp_p.tile([128, 1], F32, tag="var", name="var")
            nc.vector.scalar_tensor_tensor(vscr, x[:], 1.0, x[:],
                                           OP.mult, OP.mult, accum_out=var[:])
            vare = ftmp_p.tile([128, 1], F32, tag="vare", name="vare")
            nc.vector.tensor_scalar(vare[:], var[:], 1.0 / DM, float(LN_EPS),
                                    OP.mult, OP.add)
            sd = ftmp_p.tile([128, 1], F32, tag="sd", name="sd")
            nc.scalar.sqrt(sd[:], vare[:])
            rstd = ftmp_p.tile([128, 1], F32, tag="rstd", name="rstd")
            nc.vector.reciprocal(rstd[:], sd[:])
            nc.vector.scalar_tensor_tensor(x[:], x[:], rstd[:], t_lng[:],
                                           OP.mult, OP.mult)
            nc.gpsimd.tensor_add(x[:], x[:], t_lnb[:])
            nc.sync.dma_start(d_out[cs, :], x[:])

    return nc


# ---------------------------------------------------------------- host side
def _prep_core_inputs(h_b, W_qkvb, W_o, ln_g, ln_b):
    bf16 = ml_dtypes.bfloat16
    fp8 = ml_dtypes.float8_e4m3
    hT = np.ascontiguousarray(h_b.T).astype(fp8)                   # [1024, 2048]
    wq = np.zeros((DM, OTOT), dtype=np.float32)
    Wr = W_qkvb.reshape(NH, 193, DM)
    for hd in range(NH):
        wq[:, hd * 192:hd * 192 + 192] = Wr[hd, 0:192, :].T
        wq[:, OQKV + hd] = Wr[hd, 192, :]
    wq = (wq * WSC).astype(fp8)
    woT = np.ascontiguousarray(W_o.T * SCALE).astype(bf16)         # [i, o]
    lng = np.broadcast_to(ln_g[None, :], (128, DM)).astype(bf16).copy()
    lnb = np.broadcast_to(ln_b[None, :], (128, DM)).astype(bf16).copy()
    ii, jj = np.indices((128, 132))
    mSL = (jj < ii).astype(np.float32);  mSL[:, 128] = 1.0
    mLI = (jj <= ii).astype(np.float32); mLI[:, 128] = 1.0
    mUI = ((jj >= ii) & (jj < 128)).astype(np.float32)
    return {"hT": hT, "hres": np.ascontiguousarray(h_b, np.float32),
            "wqkv": wq, "woT": woT, "lng": lng, "lnb": lnb,
            "maskSL": mSL, "maskUI": mUI}


_cached = {}


def kernel(h, W_qkvb, W_o, ln_g, ln_b):
    h = np.asarray(h, np.float32)
    W_qkvb = np.asarray(W_qkvb, np.float32)
    W_o = np.asarray(W_o, np.float32)
    ln_g = np.asarray(ln_g, np.float32)
    ln_b = np.asarray(ln_b, np.float32)
    if "nc" not in _cached:
        _cached["nc"] = build_program()
    nc = _cached["nc"]
    in_maps = [_prep_core_inputs(h[:, b, :], W_qkvb, W_o, ln_g, ln_b)
               for b in range(BSZ)]
    res = run_bass_kernel_spmd(nc, in_maps, list(range(BSZ)),
                               trace=os.environ.get("BASS_TRACE", "") == "1")
    out = np.stack([res.results[b]["out"] for b in range(BSZ)], axis=1)
    kernel.last_exec_time_ns = res.exec_time_ns
    return out.astype(np.float32)


# revision 5
# speedup vs baseline: 1.2449x; 1.0289x over previous
"""Trainium2 Bass kernel v2: head-batched CudaNorm FastWeight DPFP layer.

Batch sharded across 8 cores (1 batch element per core). Per chunk (C=128):
qkvb projection (PSUM-wide groups), DPFP features batched over all 16 heads
via strided 3D APs, merged Gram matmuls ([A|kr|S1] + [S2|qr] per head in one
PSUM bank), batched mask/reduce extraction, 3-matmul Neumann solve per head
(Y = (I-N)(I+N^2)X0 via t1=NX0, t2=N*t1, pY=N*X1), transpose-free Shat,
single all-heads W state with one Pool copy per chunk.
"""
import os
import numpy as np
import ml_dtypes

import concourse.bass as bass
import concourse.mybir as mybir
from concourse.bass_utils import run_bass_kernel_spmd
from concourse.tile import TileContext
from concourse.vector_clock import ScopedClock, VectorClock
from contextlib import ExitStack

F32 = mybir.dt.float32
BF16 = mybir.dt.bfloat16
FP8 = mybir.dt.float8e4
WSC = 32.0
AF = mybir.ActivationFunctionType
OP = mybir.AluOpType
AX = mybir.AxisListType

SLEN, BSZ, DM = 2048, 8, 1024
NH, DH, NROLL = 16, 64, 2
D = 2 * NROLL * DH            # 256 feature dim (2 dc of 128)
C = 128                       # chunk length
NCH = SLEN // C               # 16 chunks
EPS, LN_EPS = 1e-5, 1e-5
SCALE = 1.0 / float(np.sqrt(DH))
OQKV = NH * 192               # 3072
OTOT = OQKV + NH              # 3088
KQW = 260                     # per-head stride in KQ tiles: K(128)|r(1)|Q(128)|pad

MAXW = 2


class PatchedTileContext(TileContext):
    """Work around walrus TPB sync-command limits: each instruction carries at
    most 2 sync commands (waits+updates); hoist excess waits onto preceding
    same-engine NoOps (1 wait each), and emit the kernel-tail drain's waits
    one-per-nop on SP."""

    def _lower_ordered_insts(self, ordered):
        for bb_name in list(ordered.keys()):
            new = []
            for inst in ordered[bb_name]:
                si = inst.sync_info
                nupd = len(si.on_update) if si is not None and si.on_update else 0
                maxw = max(0, MAXW - nupd)
                if si is not None and si.on_wait and len(si.on_wait) > maxw:
                    waits = list(si.on_wait)
                    excess = waits if maxw == 0 else waits[:-maxw]
                    keep = [] if maxw == 0 else waits[-maxw:]
                    for w in excess:
                        nop = mybir.InstNoOp(
                            name=self.nc.get_next_instruction_name(),
                            engine=inst.engine, ins=[], outs=[])
                        nop.sync_info = mybir.SyncInfo(on_wait=[w], on_update=[])
                        new.append(nop)
                    inst.sync_info = mybir.SyncInfo(
                        on_wait=keep, on_update=list(si.on_update or []))
                new.append(inst)
            ordered[bb_name] = new
        return super()._lower_ordered_insts(ordered)

    def _drain_and_barrier(self, tick_clock, wait_clock):
        gc = tick_clock.global_clock
        n = len(gc)
        for p in range(n):
            if gc[p] > 0:
                vc = VectorClock([gc[i] if i == p else 0 for i in range(n)])
                nop = self.nc.sync.nop(nofuse=True)
                wait_clock.add_sem_waits(nop.ins, ScopedClock({None: vc}))
        self.nc.sync.drain()
        self.nc.all_engine_barrier()
        assert self.sems is not None
        popped = self.nc._tile_sem_poison_stack.pop()
        assert popped is self._sem_poison
        self.nc.clear_and_free_semaphores(list(self.sems.allocated().values()))
        self.nc.all_engine_barrier()


def _r3(ap, h):
    return ap.rearrange("p (h o) -> p h o", h=h)


def _b3(ap, n, w):
    # [128, n] -> [128, n, w] with stride-0 inner axis
    return ap.unsqueeze(-1).broadcast_to([128, n, w])


# ---------------------------------------------------------------- program
def build_program(n_chunks=NCH):
    nc = bass.Bass()
    d_hT = nc.declare_dram_parameter("hT", [DM, SLEN], FP8, isOutput=False)
    d_hres = nc.declare_dram_parameter("hres", [SLEN, DM], F32, isOutput=False)
    d_w = nc.declare_dram_parameter("wqkv", [DM, OTOT], FP8, isOutput=False)
    d_wo = nc.declare_dram_parameter("woT", [DM, DM], BF16, isOutput=False)
    d_lng = nc.declare_dram_parameter("lng", [128, DM], BF16, isOutput=False)
    d_lnb = nc.declare_dram_parameter("lnb", [128, DM], BF16, isOutput=False)
    d_mAS = nc.declare_dram_parameter("maskAS", [128, 257], F32, isOutput=False)
    d_out = nc.declare_dram_parameter("out", [SLEN, DM], F32, isOutput=True)

    with PatchedTileContext(nc) as tc, ExitStack() as ctx:
        P = lambda name, bufs, **kw: ctx.enter_context(
            tc.tile_pool(name=name, bufs=bufs, **kw))
        const = P("const", 1)
        state = P("state", 1)
        hts_p = P("hts", 1)
        raw_p = P("raw", 1)
        sig_p = P("sig", 1)
        fx_p = P("fx", 1)      # one [128,2048] bf16 tag, rotated q/k
        ff_p = P("ff", 1)      # one [128,2048] f32 tag, rotated 4x (q/k halves)
        fn_p = P("fn", 2)
        kq_p = P("kq", 2)
        scr_p = P("scr", 1)
        cols_p = P("cols", 1)
        sc_p = P("sc", 1)
        oc_p = P("oc", 1)
        ftmp_p = P("ftmp", 4)
        xln_p = P("xln", 1)
        psG_p = P("psG", 1, space="PSUM")   # 4 banks: gram groups / projection
        psS_p = P("psS", 4, space="PSUM")   # 4 banks: solve/outproj/pW

        # ---- constants
        t_mAS = const.tile([128, 257], F32, tag="mAS", name="mAS"); nc.sync.dma_start(t_mAS[:], d_mAS[:])
        t_lng = const.tile([128, DM], BF16, tag="lng", name="lng"); nc.sync.dma_start(t_lng[:], d_lng[:])
        t_lnb = const.tile([128, DM], BF16, tag="lnb", name="lnb"); nc.sync.dma_start(t_lnb[:], d_lnb[:])
        tw_all = const.tile([128, 8 * OTOT], FP8, tag="tw", name="tw")
        nc.sync.dma_start(
            _r3(tw_all[:], 8),
            d_w[:].rearrange("(mc p) o -> mc p o", mc=8).transpose([1, 0, 2]))
        tw4 = _r3(tw_all[:], 8)
        t_wo = []
        for ic in range(8):
            t = const.tile([128, DM], BF16, tag=f"wo{ic}", name=f"wo{ic}")
            nc.sync.dma_start(t[:], d_wo[ic * 128:(ic + 1) * 128, :])
            t_wo.append(t)

        # ---- state (Wb per head: [Wdc0(64) | r0(1) | Wdc1(64) | r1(1)])
        t_Wm = state.tile([128, NH * 128], F32, tag="Wm", name="Wm")
        nc.vector.memset(t_Wm[:], 0.0)
        t_Wb = state.tile([128, NH * 130], BF16, tag="Wb", name="Wb")
        nc.vector.memset(t_Wb[:], 0.0)
        t_r = []
        for dc in range(2):
            r = state.tile([128, NH], F32, tag=f"r{dc}", name=f"r{dc}")
            nc.vector.memset(r[:], 0.0)
            t_r.append(r)

        for c in range(n_chunks):
            cs = slice((c % NCH) * 128, (c % NCH) * 128 + 128)
            # ================= projection =================================
            hts = hts_p.tile([128, 1024], FP8, tag="hts", name="hts")
            src = d_hT[:, cs].rearrange("(mc p) t -> mc p t", mc=8).transpose([1, 0, 2])
            nc.sync.dma_start(_r3(hts[:], 8), src)
            raw = raw_p.tile([128, OTOT], BF16, tag="raw", name="raw")
            pg = psG_p.tile([128, 2048], F32, tag="G", name="G")
            for g in range(4):
                for m2 in range(4):
                    nc.tensor.matmul(
                        pg[:, g * 512:(g + 1) * 512],
                        hts[:, m2 * 256:(m2 + 1) * 256].rearrange(
                            "p (two f) -> p two f", two=2),
                        tw4[:, 2 * m2:2 * m2 + 2, g * 512:(g + 1) * 512],
                        start=(m2 == 0), stop=(m2 == 3),
                        perf_mode=mybir.MatmulPerfMode.DoubleRow)
            nc.vector.tensor_scalar_mul(raw[:, 0:2048], pg[:], 1.0 / WSC)
            pg2 = psG_p.tile([128, 2048], F32, tag="G", name="G")
            for gi, (o0, ow) in enumerate([(2048, 512), (2560, 512), (OQKV, NH)]):
                for m2 in range(4):
                    nc.tensor.matmul(
                        pg2[:, gi * 512:gi * 512 + ow],
                        hts[:, m2 * 256:(m2 + 1) * 256].rearrange(
                            "p (two f) -> p two f", two=2),
                        tw4[:, 2 * m2:2 * m2 + 2, o0:o0 + ow],
                        start=(m2 == 0), stop=(m2 == 3),
                        perf_mode=mybir.MatmulPerfMode.DoubleRow)
            nc.vector.tensor_scalar_mul(
                _r3(raw[:, 2048:OTOT].rearrange("p (a b) -> p a b", a=1)
                    .squeeze(1), 1),
                pg2[:, 0:1040].rearrange("p (a b) -> p a b", a=1).squeeze(1),
                1.0 / WSC)
            sig = sig_p.tile([128, NH], F32, tag="sig", name="sig")
            nc.scalar.activation(sig[:], raw[:, OQKV:OQKV + NH], AF.Sigmoid)

            # ================= features (all heads, strided) ==============
            # fqn/fkn are dc-major: [128, (2 dc, 16 h, 128)] so each dc block
            # is contiguous and transposes in ONE batched DMA transpose.
            rawq = _r3(raw[:, 0:OQKV], NH)  # [128, 16, 192]
            fqn = fn_p.tile([128, NH * 256], BF16, tag="fqn", name="fqn")
            fkn = fn_p.tile([128, NH * 256], BF16, tag="fkn", name="fkn")
            for which, (coff, fdst) in enumerate(((0, fqn), (64, fkn))):
                eng = nc.vector
                xp = fx_p.tile([128, NH * 128], BF16, tag="xp", name="xp")
                x3 = _r3(xp[:], NH)
                nc.scalar.activation(x3[:, :, 0:64], rawq[:, :, coff:coff + 64],
                                     AF.Relu)
                nc.scalar.activation(x3[:, :, 64:128],
                                     rawq[:, :, coff:coff + 64], AF.Relu,
                                     scale=-1.0)
                sums = ftmp_p.tile([128, NH], F32, tag=f"sums{which}",
                                   name=f"sums{which}")
                for half in range(2):
                    hh = slice(half * 8, (half + 1) * 8)
                    f = ff_p.tile([128, 8 * 256], BF16, tag="f", name="f")
                    # f dc-major: [128, (2 dc, 8 h, 128)]
                    f4 = f[:].rearrange("p (d h o) -> p d h o", d=2, h=8)
                    xh = x3[:, hh, :]
                    eng.tensor_mul(f4[:, 0, :, 1:128], xh[:, :, 1:128],
                                   xh[:, :, 0:127])
                    eng.tensor_mul(f4[:, 0, :, 0:1], xh[:, :, 0:1],
                                   xh[:, :, 127:128])
                    eng.tensor_mul(f4[:, 1, :, 2:128], xh[:, :, 2:128],
                                   xh[:, :, 0:126])
                    eng.tensor_mul(f4[:, 1, :, 0:2], xh[:, :, 0:2],
                                   xh[:, :, 126:128])
                    sr = f[:].rearrange("p (d h o) -> p h d o", d=2, h=8)
                    nc.vector.tensor_reduce(sums[:, hh], sr, AX.XY, OP.add)
                    rec = ftmp_p.tile([128, 8], F32, tag=f"rec{which}{half}",
                                      name=f"rec{which}{half}")
                    nc.vector.reciprocal(rec[:], sums[:, hh])
                    fd4 = fdst[:].rearrange("p (d h o) -> p d h o", d=2, h=NH)
                    recb = rec[:].unsqueeze(1).unsqueeze(-1).broadcast_to(
                        [128, 2, 8, 128])
                    eng.tensor_mul(fd4[:, :, hh, :], f4, recb)

            # ---- transposes into KQ tiles: per head [K(128)|r(1)|Q(128)|pad]
            KQ = [kq_p.tile([128, NH * KQW], BF16, tag=f"KQ{dc}", name=f"KQ{dc}")
                  for dc in range(2)]
            for dc in range(2):
                for which, fsrc, off in ((0, fkn, 0), (1, fqn, 129)):
                    stage = ff_p.tile([128, 2048], BF16, tag="stg", name="stg")
                    eng = nc.sync if (dc + which) % 2 == 0 else nc.scalar
                    eng.dma_start_transpose(
                        _r3(stage[:], NH),
                        fsrc[:, dc * NH * 128:(dc + 1) * NH * 128])
                    nc.scalar.copy(_r3(KQ[dc][:], NH)[:, :, off:off + 128],
                                   _r3(stage[:], NH))
                nc.vector.tensor_copy(
                    _r3(KQ[dc][:], NH)[:, :, 128:129],
                    t_r[dc][:].unsqueeze(-1))

            # ================= gram + extraction ==========================
            kd_all = cols_p.tile([128, NH], F32, tag="kd", name="kd")
            AS = scr_p.tile([128, NH * 257], BF16, tag="AS", name="AS")
            for g in range(4):
                pgr = psG_p.tile([128, 2048], F32, tag="G", name="G")
                for j in range(4):
                    hd = g * 4 + j
                    base = hd * KQW
                    for dc in range(2):
                        nc.tensor.matmul(pgr[:, j * 512:j * 512 + 257],
                                         KQ[dc][:, base:base + 128],
                                         KQ[dc][:, base:base + 257],
                                         start=(dc == 0), stop=(dc == 1))
                pgr3 = _r3(pgr[:], 4)
                AS3 = _r3(AS[:], NH)[:, g * 4:(g + 1) * 4, :]
                nc.vector.tensor_mul(
                    AS3, pgr3[:, :, 0:257],
                    t_mAS[:].unsqueeze(1).broadcast_to([128, 4, 257]))
                nc.vector.tensor_reduce(kd_all[:, g * 4:(g + 1) * 4],
                                        AS3[:, :, 0:129], AX.X, OP.add)
            if c == 0:
                nc.vector.memset(kd_all[0:1, :], 1.0)

            # ================= chunk columns ==============================
            ceps = cols_p.tile([128, NH], F32, tag="ceps", name="ceps")
            nc.vector.tensor_scalar_add(ceps[:], kd_all[:], EPS)
            c_all = cols_p.tile([128, NH], F32, tag="c", name="c")
            nc.vector.reciprocal(c_all[:], ceps[:])
            t0 = cols_p.tile([128, NH], F32, tag="t0", name="t0")
            nc.vector.tensor_mul(t0[:], kd_all[:], c_all[:])
            cb_all = cols_p.tile([128, NH], F32, tag="cb", name="cb")
            nc.vector.tensor_mul(cb_all[:], t0[:], sig[:])
            cbc = cols_p.tile([128, NH], F32, tag="cbc", name="cbc")
            nc.vector.tensor_mul(cbc[:], cb_all[:], c_all[:])

            # ---- Nt (all heads) + Bt transposes
            Nt = scr_p.tile([128, NH * 128], BF16, tag="Nt", name="Nt")
            nc.vector.tensor_mul(_r3(Nt[:], NH),
                                 _r3(AS[:], NH)[:, :, 0:128],
                                 _b3(cbc[:], NH, 128))
            Bt = scr_p.tile([128, NH * 128], BF16, tag="Bt", name="Bt")
            nc.sync.dma_start_transpose(_r3(Bt[:], NH), Nt[:])

            # ================= solve + outputs + W ========================
            outc = oc_p.tile([128, NH * 64], BF16, tag="outc", name="outc")
            vcb = oc_p.tile([128, NH * 64], BF16, tag="vall", name="vall")
            nc.vector.tensor_mul(_r3(vcb[:], NH), rawq[:, :, 128:192],
                                 _b3(cb_all[:], NH, 64))
            Yts = []
            for g8 in range(2):
                hs = slice(g8 * 8, (g8 + 1) * 8)
                if c > 0:
                    pkw = psS_p.tile([128, 512], F32, tag="S", name="S")
                    for j in range(8):
                        hd = g8 * 8 + j
                        for dc in range(2):
                            nc.tensor.matmul(
                                pkw[:, j * 64:(j + 1) * 64],
                                KQ[dc][:, hd * KQW:hd * KQW + 128],
                                t_Wb[:, hd * 130 + dc * 65:hd * 130 + dc * 65 + 64],
                                start=(dc == 0), stop=(dc == 1))
                    xf = sc_p.tile([128, 512], F32, tag="xf", name="xf")
                    nc.vector.tensor_mul(_r3(xf[:], 8), _r3(pkw[:], 8),
                                         _b3(cbc[:, hs], 8, 64))
                    X0g = sc_p.tile([128, 512], BF16, tag=f"X0{g8}",
                                    name=f"X0{g8}")
                    nc.vector.tensor_sub(
                        X0g[:], vcb[:, g8 * 512:(g8 + 1) * 512], xf[:])
                X0ap = (vcb[:, g8 * 512:(g8 + 1) * 512] if c == 0
                        else X0g[:])
                pt1 = psS_p.tile([128, 512], F32, tag="S", name="S")
                for j in range(8):
                    hd = g8 * 8 + j
                    nc.tensor.matmul(pt1[:, j * 64:(j + 1) * 64],
                                     Bt[:, hd * 128:(hd + 1) * 128],
                                     X0ap[:, j * 64:(j + 1) * 64],
                                     start=True, stop=True)
                t1s = sc_p.tile([128, 512], BF16, tag=f"t1s{g8}", name=f"t1s{g8}")
                nc.vector.tensor_copy(t1s[:], pt1[:])
                pt2 = psS_p.tile([128, 512], F32, tag="S", name="S")
                for j in range(8):
                    hd = g8 * 8 + j
                    nc.tensor.matmul(pt2[:, j * 64:(j + 1) * 64],
                                     Bt[:, hd * 128:(hd + 1) * 128],
                                     t1s[:, j * 64:(j + 1) * 64],
                                     start=True, stop=True)
                X1g = sc_p.tile([128, 512], BF16, tag=f"X1{g8}", name=f"X1{g8}")
                nc.vector.tensor_add(X1g[:], pt2[:], X0ap)
                py = psS_p.tile([128, 512], F32, tag="S", name="S")
                for j in range(8):
                    hd = g8 * 8 + j
                    nc.tensor.matmul(py[:, j * 64:(j + 1) * 64],
                                     Bt[:, hd * 128:(hd + 1) * 128],
                                     X1g[:, j * 64:(j + 1) * 64],
                                     start=True, stop=True)
                Ytg = sc_p.tile([128, 8 * 65], BF16, tag=f"Yt{g8}", name=f"Yt{g8}")
                Yt3 = _r3(Ytg[:], 8)
                nc.vector.tensor_sub(Yt3[:, :, 0:64], _r3(X1g[:], 8),
                                     _r3(py[:], 8))
                if c == 0:
                    nc.vector.memset(Yt3[:, :, 64:65], 1.0)
                Yts.append(Ytg)

            # ---- pOut (4-head groups; col 64 accumulates qr + Shat colsum = dn)
            for g in range(4):
                pout = psS_p.tile([128, 4 * 65], F32, tag="S", name="S")
                Ytg = Yts[g // 2]
                for j in range(4):
                    hd = g * 4 + j
                    jj = hd % 8
                    base = hd * KQW
                    if c == 0:
                        nc.tensor.matmul(pout[:, j * 65:(j + 1) * 65],
                                         AS[:, hd * 257 + 129:hd * 257 + 257],
                                         Ytg[:, jj * 65:(jj + 1) * 65],
                                         start=True, stop=True)
                    else:
                        for dc in range(2):
                            nc.tensor.matmul(
                                pout[:, j * 65:(j + 1) * 65],
                                KQ[dc][:, base + 129:base + 257],
                                t_Wb[:, hd * 130 + dc * 65:hd * 130 + (dc + 1) * 65],
                                start=(dc == 0), stop=False)
                        nc.tensor.matmul(pout[:, j * 65:(j + 1) * 65],
                                         AS[:, hd * 257 + 129:hd * 257 + 257],
                                         Ytg[:, jj * 65:(jj + 1) * 65],
                                         start=False, stop=True)
                pout3 = _r3(pout[:], 4)
                dnrg = ftmp_p.tile([128, 4], F32, tag="dnrg", name="dnrg")
                nc.vector.reciprocal(dnrg[:].unsqueeze(-1),
                                     pout3[:, :, 64:65])
                nc.vector.tensor_mul(
                    _r3(outc[:], NH)[:, g * 4:(g + 1) * 4, :],
                    pout3[:, :, 0:64], _b3(dnrg[:], 4, 64))

            # ---- W update (groups of 4 heads)
            for g in range(4):
                pw = psS_p.tile([128, 512], F32, tag="S", name="S")
                for j in range(4):
                    hd = g * 4 + j
                    for dc in range(2):
                        nc.tensor.matmul(
                            pw[:, j * 128 + dc * 64:j * 128 + (dc + 1) * 64],
                            fkn[:, dc * NH * 128 + hd * 128:
                                dc * NH * 128 + (hd + 1) * 128],
                            Yts[hd // 8][:, (hd % 8) * 65:(hd % 8) * 65 + 64],
                            start=True, stop=True)
                nc.vector.tensor_add(t_Wm[:, g * 512:(g + 1) * 512], pw[:],
                                     t_Wm[:, g * 512:(g + 1) * 512])
            Wb3 = t_Wb[:].rearrange("p (h o) -> p h o", h=NH)
            Wm3 = _r3(t_Wm[:], NH)
            for dc in range(2):
                nc.vector.tensor_copy(Wb3[:, :, dc * 65:dc * 65 + 64],
                                      Wm3[:, :, dc * 64:(dc + 1) * 64])
            # ---- r update (also lands in Wb cols 64/129 for the dn fold)
            for dc in range(2):
                rs = ftmp_p.tile([128, NH], F32, tag=f"rs{dc}", name=f"rs{dc}")
                nc.vector.tensor_reduce(rs[:], _r3(KQ[dc][:], NH)[:, :, 0:128],
                                        AX.X, OP.add)
                nc.gpsimd.tensor_add(t_r[dc][:], t_r[dc][:], rs[:])
                nc.vector.tensor_copy(Wb3[:, :, dc * 65 + 64:dc * 65 + 65],
                                      t_r[dc][:].unsqueeze(-1))

            # ================= outT + out-proj + residual + LN ============
            outT = oc_p.tile([128, NH * 64], BF16, tag="outT", name="outT")
            nc.scalar.dma_start_transpose(_r3(outT[:], 8), outc[:])
            hr = xln_p.tile([128, DM], F32, tag="hr", name="hr")
            nc.sync.dma_start(hr[:], d_hres[cs, :])
            x = xln_p.tile([128, DM], F32, tag="x", name="x")
            for og in range(2):
                pat = psS_p.tile([128, 512], F32, tag="S", name="S")
                for ic in range(8):
                    nc.tensor.matmul(pat[:], outT[:, ic * 128:(ic + 1) * 128],
                                     t_wo[ic][:, og * 512:(og + 1) * 512],
                                     start=(ic == 0), stop=(ic == 7))
                nc.vector.tensor_add(x[:, og * 512:(og + 1) * 512], pat[:],
                                     hr[:, og * 512:(og + 1) * 512])
            xsum = ftmp_p.tile([128, 1], F32, tag="xsum", name="xsum")
            nc.vector.tensor_reduce(xsum[:], x[:], AX.X, OP.add)
            nmu = ftmp_p.tile([128, 1], F32, tag="nmu", name="nmu")
            nc.vector.tensor_scalar_mul(nmu[:], xsum[:], -1.0 / DM)
            nc.gpsimd.tensor_scalar_add(x[:], x[:], nmu[:])
            vscr = hr[:].bitcast(BF16)[:, 0:DM]
            var = ftmp_p.tile([128, 1], F32, tag="var", name="var")
            nc.vector.scalar_tensor_tensor(vscr, x[:], 1.0, x[:],
                                           OP.mult, OP.mult, accum_out=var[:])
            vare = ftmp_p.tile([128, 1], F32, tag="vare", name="vare")
            nc.vector.tensor_scalar(vare[:], var[:], 1.0 / DM, float(LN_EPS),
                                    OP.mult, OP.add)
            sd = ftmp_p.tile([128, 1], F32, tag="sd", name="sd")
            nc.scalar.sqrt(sd[:], vare[:])
            rstd = ftmp_p.tile([128, 1], F32, tag="rstd", name="rstd")
            nc.vector.reciprocal(rstd[:], sd[:])
            nc.vector.scalar_tensor_tensor(x[:], x[:], rstd[:], t_lng[:],
                                           OP.mult, OP.mult)
            nc.gpsimd.tensor_add(x[:], x[:], t_lnb[:])
            nc.sync.dma_start(d_out[cs, :], x[:])

    return nc


# ---------------------------------------------------------------- host side
def _prep_core_inputs(h_b, W_qkvb, W_o, ln_g, ln_b):
    bf16 = ml_dtypes.bfloat16
    fp8 = ml_dtypes.float8_e4m3
    hT = np.ascontiguousarray(h_b.T).astype(fp8)                   # [1024, 2048]
    wq = np.zeros((DM, OTOT), dtype=np.float32)
    Wr = W_qkvb.reshape(NH, 193, DM)
    for hd in range(NH):
        wq[:, hd * 192:hd * 192 + 192] = Wr[hd, 0:192, :].T
        wq[:, OQKV + hd] = Wr[hd, 192, :]
    wq = (wq * WSC).astype(fp8)
    woT = np.ascontiguousarray(W_o.T * SCALE).astype(bf16)         # [i, o]
    lng = np.broadcast_to(ln_g[None, :], (128, DM)).astype(bf16).copy()
    lnb = np.broadcast_to(ln_b[None, :], (128, DM)).astype(bf16).copy()
    ii, jj = np.indices((128, 257))
    mAS = (jj < ii).astype(np.float32)          # strict lower for A
    mAS[:, 128] = 1.0                           # keep kr column
    mAS[:, 129:257] = (jj[:, 129:257] - 129 >= ii[:, 129:257]).astype(np.float32)
    return {"hT": hT, "hres": np.ascontiguousarray(h_b, np.float32),
            "wqkv": wq, "woT": woT, "lng": lng, "lnb": lnb,
            "maskAS": mAS.astype(np.float32)}


_cached = {}


def kernel(h, W_qkvb, W_o, ln_g, ln_b):
    h = np.asarray(h, np.float32)
    W_qkvb = np.asarray(W_qkvb, np.float32)
    W_o = np.asarray(W_o, np.float32)
    ln_g = np.asarray(ln_g, np.float32)
    ln_b = np.asarray(ln_b, np.float32)
    if "nc" not in _cached:
        _cached["nc"] = build_program()
    nc = _cached["nc"]
    in_maps = [_prep_core_inputs(h[:, b, :], W_qkvb, W_o, ln_g, ln_b)
               for b in range(BSZ)]
    res = run_bass_kernel_spmd(nc, in_maps, list(range(BSZ)),
                               trace=os.environ.get("BASS_TRACE", "") == "1")
    out = np.stack([res.results[b]["out"] for b in range(BSZ)], axis=1)
    kernel.last_exec_time_ns = res.exec_time_ns
    return out.astype(np.float32)
